# revision 1
# baseline (speedup 1.0000x reference)
"""Trainium2 Bass kernel for masked multi-head attention (8-core SPMD).

Problem: B=2, S=2048, d_in=hid=512, H=8 heads (dh=64), fp32.
Reference quirk: the mask uses np.tile(valid_length, H), so scores row
i = b*H + h is masked with valid_length[(b*H + h) % 2] = vl[h % 2] —
the mask depends on HEAD PARITY, not batch. Even heads use vl[0], odd
heads vl[1], in both batches.

Sharding (8 cores): core c = (batch b = c//4, head-pair p = c%4).
Each core computes heads {2p, 2p+1} of batch b over the full 2048
queries, producing its partial output [2048, 512] (through its 128
rows of Wo). Host sums the 4 pair-partials per batch (pure unshard).
Load is balanced by construction: every core has one even (long mask)
and one odd (short mask) head.

Per-core dataflow ("layout 2": scoresT = [keys, queries]):
  host-pretransposed inputs qT/kT/vT [512, *]; per head h (local l,
  nkt_l key tiles of 128 from vl[h%2]):
  - qT2h [128=2*64, 2048] = Wq_p^T @ queryT          (PE)
  - kT/vT rows l*64..: per-head col-block projections over its K range
  - vT PE-transposed per key tile into v_aug [128 keys, kt, l, 65]
    (col 64 = ones -> the PV matmul emits the softmax denominator)
  - per (l, q-chunk 512, key tile): scoresT [128, 512] = kT^T @ qT2h;
    exp on ACT with scale=1/8, per-partition mask bias on the boundary
    key tile only (exp(-1e9) == 0.0 exactly, matching the reference's
    masked softmax terms; no max-subtraction needed since scores are
    ~N(0,1) after the 1/8 scale); PV: [65, 512] += v_aug^T @ expS
  - normalize via reciprocal + rank-1 broadcast matmul + multiply
  - Wo: partial[q,512] = outT2h^T @ Wo_p rows; DMA out.
"""

import math

import numpy as np

import concourse.bass as bass
from concourse import bacc
import concourse.mybir as mybir
import concourse.tile as tile
from concourse.bass_utils import run_bass_kernel_spmd
from concourse.masks import make_identity

F32 = mybir.dt.float32
F32R = mybir.dt.float32r
EXP = mybir.ActivationFunctionType.Exp

B, S, D, HID, H, DH = 2, 2048, 512, 512, 8, 64
NQC = S // 512       # q chunks
MASK_BIAS = -1e9     # exp(x + -1e9) == 0.0 in fp32

USE_F32R = False     # fp32r: 4x faster PE streaming, ~tf32 precision


def _r(ap):
    return ap.bitcast(F32R) if USE_F32R else ap


def _build(nkt_e: int, nkt_o: int):
    """One BIR program, same on all 8 cores. nkt_e/nkt_o = number of
    128-key tiles for the even/odd head (from vl[0]/vl[1])."""
    nc = bacc.Bacc("TRN2", target_bir_lowering=False, debug=False,
                   num_devices=8)
    NKT = (nkt_e, nkt_o)
    KMAX = max(NKT) * 128
    NKTM = max(NKT)

    qT_d = nc.dram_tensor("qT", [4, 128, S], F32, kind="ExternalInput").ap()
    kT_d = nc.dram_tensor("kT", [4, 128, KMAX], F32, kind="ExternalInput").ap()
    vT_d = nc.dram_tensor("vT", [4, 128, KMAX], F32, kind="ExternalInput").ap()
    wq_d = nc.dram_tensor("wq", [4, 128, 128], F32, kind="ExternalInput").ap()
    wk_d = nc.dram_tensor("wk", [4, 128, 128], F32, kind="ExternalInput").ap()
    wv_d = nc.dram_tensor("wv", [4, 128, 128], F32, kind="ExternalInput").ap()
    wo_d = nc.dram_tensor("wo", [128, 512], F32, kind="ExternalInput").ap()
    mask_d = [nc.dram_tensor(f"mask{l}", [128, 1], F32, kind="ExternalInput").ap()
              for l in range(2)]
    out_d = nc.dram_tensor("out", [S // 128, 128, 512], F32,
                           kind="ExternalOutput").ap()

    with tile.TileContext(nc) as tc:
        with (
            tc.tile_pool(name="consts", bufs=1) as consts,
            tc.tile_pool(name="inputs", bufs=1) as inputs,
            tc.tile_pool(name="work", bufs=1) as work,
            tc.tile_pool(name="exps", bufs=4) as exps,
            tc.tile_pool(name="small", bufs=4) as small,
            tc.tile_pool(name="pst", bufs=2, space="PSUM") as pst,
            tc.tile_pool(name="pssc", bufs=3, space="PSUM") as pssc,
            tc.tile_pool(name="pspv", bufs=2, space="PSUM") as pspv,
        ):
            ident = consts.tile([128, 128], F32)
            make_identity(nc, ident[:])
            ones = consts.tile([1, 64], F32)
            nc.vector.memset(ones[:], 1.0)
            wq_s = consts.tile([128, 4, 128], F32)
            wk_s = consts.tile([128, 4, 128], F32)
            wv_s = consts.tile([128, 4, 128], F32)
            wo_s = consts.tile([128, 512], F32)
            for dt in range(4):
                nc.sync.dma_start(wq_s[:, dt], wq_d[dt])
                nc.sync.dma_start(wk_s[:, dt], wk_d[dt])
                nc.sync.dma_start(wv_s[:, dt], wv_d[dt])
            nc.sync.dma_start(wo_s[:], wo_d[:])
            mask_s = []
            for l in range(2):
                m = consts.tile([128, 1], F32, tag=f"mask{l}")
                nc.sync.dma_start(m[:], mask_d[l][:])
                mask_s.append(m)

            qT_in = inputs.tile([128, 4, S], F32)
            kT_in = inputs.tile([128, 4, KMAX], F32)
            vT_in = inputs.tile([128, 4, KMAX], F32)
            for dt in range(4):
                nc.sync.dma_start(kT_in[:, dt], kT_d[dt])
                nc.sync.dma_start(qT_in[:, dt], qT_d[dt])
                nc.sync.dma_start(vT_in[:, dt], vT_d[dt])

            # ---- projections ----
            qT = work.tile([128, S], F32)      # [2*64 cols, q]
            kT = work.tile([128, KMAX], F32)   # rows l*64.., keys
            vT = work.tile([128, KMAX], F32)

            for c in range(NQC):   # qT2h: both heads at once
                ps = pst.tile([128, 512], F32, tag="t", name="pq")
                for dt in range(4):
                    nc.tensor.matmul(ps[:], _r(wq_s[:, dt]),
                                     _r(qT_in[:, dt, c * 512:(c + 1) * 512]),
                                     start=(dt == 0), stop=(dt == 3))
                nc.vector.tensor_copy(qT[:, c * 512:(c + 1) * 512], ps[:])

            def proj_head(dst, w_s, src, l, K):
                # dst[l*64:(l+1)*64, :K] = W[:, l-half]^T @ src[:, :K]
                for pos in range(0, K, 512):
                    ncols = min(512, K - pos)
                    ps = pst.tile([128, 512], F32, tag="t", name="pkv")
                    o = ps[l * 64:(l + 1) * 64, :ncols]
                    for dt in range(4):
                        nc.tensor.matmul(
                            o, _r(w_s[:, dt, l * 64:(l + 1) * 64]),
                            _r(src[:, dt, pos:pos + ncols]),
                            start=(dt == 0), stop=(dt == 3),
                            tile_position=(0, l * 64))
                    nc.vector.tensor_copy(
                        dst[l * 64:(l + 1) * 64, pos:pos + ncols], o)

            for l in range(2):
                proj_head(kT, wk_s, kT_in, l, NKT[l] * 128)
                proj_head(vT, wv_s, vT_in, l, NKT[l] * 128)

            # ---- v_aug [128, NKTM, 2, 65] via PE transposes ----
            vaug = work.tile([128, NKTM, 2, 65], F32)
            nc.vector.memset(vaug[:, :, :, 64], 1.0)
            for kt in range(NKTM):
                tp = pst.tile([128, 512], F32, tag="t", name="ptp")
                nc.tensor.transpose(tp[:, 0:128], vT[:, kt * 128:(kt + 1) * 128],
                                    ident[:])
                live = [l for l in range(2) if kt < NKT[l]]
                if len(live) == 2:
                    nc.vector.tensor_copy(
                        vaug[:, kt, :, 0:64],
                        tp[:, 0:128].rearrange("p (h d) -> p h d", h=2))
                else:
                    l = live[0]
                    nc.vector.tensor_copy(
                        vaug[:, kt, l, 0:64],
                        tp[:, l * 64:(l + 1) * 64])

            # ---- attention per (head, q-chunk) ----
            outT = work.tile([128, S], F32)
            for qc in range(NQC):
                for l in range(2):
                    nkt = NKT[l]
                    qs = qT[l * 64:(l + 1) * 64, qc * 512:(qc + 1) * 512]
                    pv = pspv.tile([65, 512], F32, tag="pv", name="pv")
                    for kt in range(nkt):
                        sc = pssc.tile([128, 512], F32, tag="sc", name="sc")
                        nc.tensor.matmul(
                            sc[:],
                            _r(kT[l * 64:(l + 1) * 64, kt * 128:(kt + 1) * 128]),
                            _r(qs), start=True, stop=True)
                        es = exps.tile([128, 512], F32, tag="es", name="es")
                        bias = mask_s[l][:] if kt == nkt - 1 else 0.0
                        nc.scalar.activation(es[:], sc[:], EXP,
                                             bias=bias, scale=0.125)
                        nc.tensor.matmul(pv[:], _r(vaug[:, kt, l, :]), _r(es[:]),
                                         start=(kt == 0), stop=(kt == nkt - 1))
                    rec = small.tile([1, 512], F32, tag="rec", name="rec")
                    nc.vector.reciprocal(rec[:], pv[64:65, :])
                    dr = pst.tile([64, 512], F32, tag="t", name="pdr")
                    nc.tensor.matmul(dr[:], ones[:], rec[:],
                                     start=True, stop=True)
                    drs = small.tile([64, 512], F32, tag="drs", name="drs")
                    nc.vector.tensor_copy(drs[:], dr[:])
                    nc.vector.tensor_mul(
                        outT[l * 64:(l + 1) * 64, qc * 512:(qc + 1) * 512],
                        pv[0:64, :], drs[:])

            # ---- output projection + store ----
            for qt in range(S // 128):
                po = pst.tile([128, 512], F32, tag="t", name="po")
                nc.tensor.matmul(po[:], _r(outT[:, qt * 128:(qt + 1) * 128]),
                                 _r(wo_s[:]), start=True, stop=True)
                so = small.tile([128, 512], F32, tag="so", name="so")
                nc.vector.tensor_copy(so[:], po[:])
                nc.sync.dma_start(out_d[qt], so[:])
    nc.compile()
    return nc


_CACHE: dict = {}


def kernel(query, key, value, Wq, Wk, Wv, Wo, valid_length):
    query = np.asarray(query); key = np.asarray(key); value = np.asarray(value)
    Wq = np.asarray(Wq, np.float32); Wk = np.asarray(Wk, np.float32)
    Wv = np.asarray(Wv, np.float32); Wo = np.asarray(Wo, np.float32)
    vl = np.asarray(valid_length).astype(np.int64)
    # head h is masked with vl[h % 2] (reference's np.tile quirk)
    nkt = [max(1, int(math.ceil(int(vl[l]) / 128))) for l in range(2)]

    key_ = (nkt[0], nkt[1])
    if key_ not in _CACHE:
        _CACHE[key_] = _build(*key_)
    nc = _CACHE[key_]
    KMAX = max(nkt) * 128

    cT = lambda a: np.ascontiguousarray(a)
    masks = []
    for l in range(2):
        base = (nkt[l] - 1) * 128
        m = np.where(base + np.arange(128) < int(vl[l]), 0.0, MASK_BIAS)
        masks.append(cT(m.reshape(128, 1).astype(np.float32)))

    in_maps = []
    for c in range(8):
        b, p = c // 4, c % 4
        im = {
            "qT": cT(query[b].T.reshape(4, 128, S)),
            "kT": cT(key[b, :KMAX].T.reshape(4, 128, KMAX)),
            "vT": cT(value[b, :KMAX].T.reshape(4, 128, KMAX)),
            "wq": cT(Wq[:, p * 128:(p + 1) * 128].reshape(4, 128, 128)),
            "wk": cT(Wk[:, p * 128:(p + 1) * 128].reshape(4, 128, 128)),
            "wv": cT(Wv[:, p * 128:(p + 1) * 128].reshape(4, 128, 128)),
            "wo": cT(Wo[p * 128:(p + 1) * 128]),
            "mask0": masks[0], "mask1": masks[1],
        }
        in_maps.append(im)

    import os
    trace = os.environ.get("BASS_KTRACE", "0") == "1"
    kw = dict(trace=True, trace_cores=list(range(8))) if trace else {}
    res = run_bass_kernel_spmd(nc, in_maps, core_ids=list(range(8)), **kw)
    kernel.last_results = res
    out = np.zeros((B, S, HID), np.float32)
    for c in range(8):
        b, p = c // 4, c % 4
        out[b] += res.results[c]["out"].reshape(S, HID)
    return out



# revision 4
# speedup vs baseline: 2.0319x; 2.0319x over previous
"""Trainium2 Bass kernel for masked multi-head attention (8-core SPMD).

Problem: B=2, S=2048, d_in=hid=512, H=8 heads (dh=64), fp32 in/out.
Reference quirk: the mask uses np.tile(valid_length, H), so scores row
i = b*H + h is masked with valid_length[h % 2] — head PARITY, not batch.

Sharding (8 cores): core c = (batch b = c//4, head-pair p = c%4).
Each core computes heads {2p, 2p+1} of batch b over all 2048 queries,
producing a partial [2048, 512] through its 128 rows of Wo; the host
sums the 4 pair-partials per batch.

Design (v3):
- bf16 on the wire and on-chip (host casts inputs); PSUM stays fp32.
  PE streams 1 cyc/row (fp32 is 4), HBM traffic halves, FWL engages.
- attention loop (query-half, head, key-tile): one kT weight load
  feeds 2 score MMs, one 1024-wide ACTIVATE per key tile, PV
  accumulates into a 2-bank PSUM tile.
- masking is baked into v_aug: columns 64:128 are ones (so PV emits
  the softmax denominator on partitions 64:128) and rows of masked
  keys are zeroed (keep vector input), so masked keys drop out of both
  numerator and denominator — no exp bias needed. exp(junk) is finite
  and multiplied by zero.
- normalize: DVE copy of the denominator rows to SBUF (partition-
  shift copies are HW-proven; reciprocal_approx_fast directly on PSUM
  at partition base 64 returned garbage on HW), then
  reciprocal_approx_fast SBUF->SBUF at base 0, then one tensor_mul.
- projection PSUM->SBUF casts run on the (otherwise idle) ScalarE.
- Wo + output DMA run per query-half, overlapping the other half's
  attention; output partial is bf16 (host sums in fp32).
"""

import math
import os

import ml_dtypes
import numpy as np

from concourse import bacc
import concourse.mybir as mybir
import concourse.tile as tile
from concourse.bass_utils import run_bass_kernel_spmd
from concourse.masks import make_identity

F32 = mybir.dt.float32
BF16 = mybir.dt.bfloat16
EXP = mybir.ActivationFunctionType.Exp

B, S, D, HID, H, DH = 2, 2048, 512, 512, 8, 64


def _build(nkt_e: int, nkt_o: int):
    """One BIR program, same on all 8 cores. nkt_e/nkt_o = number of
    128-key tiles for the even/odd head (from vl[0]/vl[1])."""
    nc = bacc.Bacc("TRN2", target_bir_lowering=False, debug=False,
                   num_devices=8)
    NKT = (nkt_e, nkt_o)
    NKTM = max(NKT)
    KMAX = NKTM * 128

    qT_d = nc.dram_tensor("qT", [4, 128, S], BF16, kind="ExternalInput").ap()
    kT_d = nc.dram_tensor("kT", [4, 128, KMAX], BF16, kind="ExternalInput").ap()
    vT_d = nc.dram_tensor("vT", [4, 128, KMAX], BF16, kind="ExternalInput").ap()
    wq_d = nc.dram_tensor("wq", [4, 128, 128], BF16, kind="ExternalInput").ap()
    wk_d = nc.dram_tensor("wk", [4, 128, 128], BF16, kind="ExternalInput").ap()
    wv_d = nc.dram_tensor("wv", [4, 128, 128], BF16, kind="ExternalInput").ap()
    wo_d = nc.dram_tensor("wo", [128, 512], BF16, kind="ExternalInput").ap()
    keep_d = [nc.dram_tensor(f"keep{l}", [128, 1], F32,
                             kind="ExternalInput").ap() for l in range(2)]
    out_d = nc.dram_tensor("out", [8, 128, 1024], BF16,
                           kind="ExternalOutput").ap()

    with tile.TileContext(nc) as tc:
        with (
            tc.tile_pool(name="consts", bufs=1) as consts,
            tc.tile_pool(name="inputs", bufs=1) as inputs,
            tc.tile_pool(name="work", bufs=1) as work,
            tc.tile_pool(name="exps", bufs=3) as exps,
            tc.tile_pool(name="recp", bufs=2) as recp,
            tc.tile_pool(name="sop", bufs=2) as sop,
        ):
            ident = consts.tile([128, 128], BF16)
            make_identity(nc, ident[:])
            wq_s = consts.tile([128, 4, 128], BF16)
            wk_s = consts.tile([128, 4, 128], BF16)
            wv_s = consts.tile([128, 4, 128], BF16)
            wo_s = consts.tile([128, 512], BF16)
            for dt in range(4):
                nc.sync.dma_start(wq_s[:, dt], wq_d[dt])
                nc.sync.dma_start(wk_s[:, dt], wk_d[dt])
                nc.sync.dma_start(wv_s[:, dt], wv_d[dt])
            nc.sync.dma_start(wo_s[:], wo_d[:])
            keep_s = []
            for l in range(2):
                m = consts.tile([128, 1], F32, tag=f"keep{l}")
                nc.sync.dma_start(m[:], keep_d[l][:])
                keep_s.append(m)

            qT_in = inputs.tile([128, 4, S], BF16)
            kT_in = inputs.tile([128, 4, KMAX], BF16)
            vT_in = inputs.tile([128, 4, KMAX], BF16)
            for dt in range(4):
                nc.sync.dma_start(qT_in[:, dt], qT_d[dt])
                nc.sync.dma_start(kT_in[:, dt], kT_d[dt])
            for dt in range(4):
                nc.sync.dma_start(vT_in[:, dt], vT_d[dt])

            qTp = work.tile([128, S], BF16)      # [2*64 head rows, q]
            kTp = work.tile([128, KMAX], BF16)   # rows l*64.., keys
            vTp = work.tile([128, KMAX], BF16)
            vaug = work.tile([128, NKTM, 2, 128], BF16)
            outT = work.tile([128, S], BF16)

            with (
                tc.tile_pool(name="pproj", bufs=3, space="PSUM") as pproj,
                tc.tile_pool(name="ptp", bufs=2, space="PSUM") as ptp,
            ):
                # ---- projections (both heads at once, fp32 psum) ----
                for c in range(S // 512):
                    ps = pproj.tile([128, 512], F32, tag="pj", name="psq")
                    for dt in range(4):
                        nc.tensor.matmul(ps[:], wq_s[:, dt],
                                         qT_in[:, dt, c * 512:(c + 1) * 512],
                                         start=(dt == 0), stop=(dt == 3))
                    nc.scalar.copy(qTp[:, c * 512:(c + 1) * 512], ps[:])
                for w_s, src, dstp in ((wk_s, kT_in, kTp), (wv_s, vT_in, vTp)):
                    for pos in range(0, KMAX, 512):
                        ncols = min(512, KMAX - pos)
                        ps = pproj.tile([128, 512], F32, tag="pj", name="pskv")
                        o = ps[:, :ncols]
                        for dt in range(4):
                            nc.tensor.matmul(o, w_s[:, dt],
                                             src[:, dt, pos:pos + ncols],
                                             start=(dt == 0), stop=(dt == 3))
                        nc.scalar.copy(dstp[:, pos:pos + ncols], o)

                # ---- v_aug [128 keys, kt, l, 64 v | 64 ones] ----
                nc.vector.memset(vaug[:, :, :, 64:128], 1.0)
                for kt in range(NKTM):
                    tp = ptp.tile([128, 128], BF16, tag="tp", name="tp")
                    nc.tensor.transpose(tp[:], vTp[:, kt * 128:(kt + 1) * 128],
                                        ident[:])
                    nc.vector.tensor_copy(
                        vaug[:, kt, :, 0:64],
                        tp[:].rearrange("p (h d) -> p h d", h=2))
                # zero masked key rows of the boundary tile (num + denom)
                for l in range(2):
                    nc.gpsimd.tensor_scalar_mul(
                        vaug[:, NKT[l] - 1, l, :], vaug[:, NKT[l] - 1, l, :],
                        keep_s[l][:])

            with (
                tc.tile_pool(name="psc", bufs=2, space="PSUM") as psc,
                tc.tile_pool(name="ppv", bufs=2, space="PSUM") as ppv,
            ):
                # ---- attention per (query-half, head); Wo per half ----
                for qh in range(2):
                    for l in range(2):
                        nkt = NKT[l]
                        pv = ppv.tile([128, 1024], F32, tag="pv", name="pv")
                        for kt in range(nkt):
                            sc = psc.tile([128, 1024], F32, tag="sc", name="sc")
                            for j in range(2):
                                qc = qh * 2 + j
                                nc.tensor.matmul(
                                    sc[:, j * 512:(j + 1) * 512],
                                    kTp[l * 64:(l + 1) * 64,
                                        kt * 128:(kt + 1) * 128],
                                    qTp[l * 64:(l + 1) * 64,
                                        qc * 512:(qc + 1) * 512],
                                    start=True, stop=True)
                            es = exps.tile([128, 1024], BF16, tag="es",
                                           name="es")
                            nc.scalar.activation(es[:], sc[:], EXP,
                                                 scale=0.125)
                            for j in range(2):
                                nc.tensor.matmul(
                                    pv[:, j * 512:(j + 1) * 512],
                                    vaug[:, kt, l, :],
                                    es[:, j * 512:(j + 1) * 512],
                                    start=(kt == 0), stop=(kt == nkt - 1))
                        dens = recp.tile([64, 1024], F32, tag="dens",
                                         name="dens")
                        nc.vector.tensor_copy(dens[:], pv[64:128, :])
                        rec = recp.tile([64, 1024], F32, tag="rec", name="rec")
                        nc.vector.reciprocal_approx_fast(rec[:], dens[:])
                        nc.vector.tensor_mul(
                            outT[l * 64:(l + 1) * 64,
                                 qh * 1024:(qh + 1) * 1024],
                            pv[0:64, :], rec[:])
                    for t in range(4):
                        po = psc.tile([128, 1024], F32, tag="sc", name="po")
                        for j in range(2):
                            qt = qh * 8 + t * 2 + j
                            nc.tensor.matmul(po[:, j * 512:(j + 1) * 512],
                                             outT[:, qt * 128:(qt + 1) * 128],
                                             wo_s[:], start=True, stop=True)
                        so = sop.tile([128, 1024], BF16, tag="so", name="so")
                        nc.vector.tensor_copy(so[:], po[:])
                        nc.sync.dma_start(out_d[qh * 4 + t], so[:])
    nc.compile()
    return nc


_CACHE: dict = {}


def kernel(query, key, value, Wq, Wk, Wv, Wo, valid_length):
    query = np.asarray(query, np.float32)
    key = np.asarray(key, np.float32)
    value = np.asarray(value, np.float32)
    Wq = np.asarray(Wq, np.float32); Wk = np.asarray(Wk, np.float32)
    Wv = np.asarray(Wv, np.float32); Wo = np.asarray(Wo, np.float32)
    vl = np.asarray(valid_length).astype(np.int64)
    # head h is masked with vl[h % 2] (reference's np.tile quirk)
    nkt = [max(1, int(math.ceil(int(vl[l]) / 128))) for l in range(2)]

    key_ = (nkt[0], nkt[1])
    if key_ not in _CACHE:
        _CACHE[key_] = _build(*key_)
    nc = _CACHE[key_]
    KMAX = max(nkt) * 128

    bf = lambda a: np.ascontiguousarray(a.astype(ml_dtypes.bfloat16))
    keeps = []
    for l in range(2):
        base = (nkt[l] - 1) * 128
        m = (base + np.arange(128) < int(vl[l])).astype(np.float32)
        keeps.append(np.ascontiguousarray(m.reshape(128, 1)))

    in_maps = []
    for c in range(8):
        b, p = c // 4, c % 4
        im = {
            "qT": bf(query[b].T).reshape(4, 128, S),
            "kT": bf(key[b, :KMAX].T).reshape(4, 128, KMAX),
            "vT": bf(value[b, :KMAX].T).reshape(4, 128, KMAX),
            "wq": bf(Wq[:, p * 128:(p + 1) * 128]).reshape(4, 128, 128),
            "wk": bf(Wk[:, p * 128:(p + 1) * 128]).reshape(4, 128, 128),
            "wv": bf(Wv[:, p * 128:(p + 1) * 128]).reshape(4, 128, 128),
            "wo": bf(Wo[p * 128:(p + 1) * 128]),
            "keep0": keeps[0], "keep1": keeps[1],
        }
        in_maps.append(im)

    trace = os.environ.get("BASS_KTRACE", "0") == "1"
    kw = dict(trace=True, trace_cores=list(range(8))) if trace else {}
    res = run_bass_kernel_spmd(nc, in_maps, core_ids=list(range(8)), **kw)
    kernel.last_results = res
    out = np.zeros((B, S, HID), np.float32)
    for c in range(8):
        b = c // 4
        r = np.asarray(res.results[c]["out"], dtype=np.float32)
        out[b] += r.reshape(8, 128, 2, 512).transpose(0, 2, 1, 3).reshape(S, HID)
    return out


# revision 7
# speedup vs baseline: 2.1687x; 1.0673x over previous
"""Trainium2 Bass kernel for masked multi-head attention (8-core SPMD).

Problem: B=2, S=2048, d_in=hid=512, H=8 heads (dh=64), fp32 in/out.
Reference quirk: the mask uses np.tile(valid_length, H), so scores row
i = b*H + h is masked with valid_length[h % 2] — head PARITY, not batch.

Sharding (8 cores): core c = (batch b = c//4, head-pair p = c%4).
Each core computes heads {2p, 2p+1} of batch b over all 2048 queries,
producing a partial [2048, 512] through its 128 rows of Wo; the host
sums the 4 pair-partials per batch.

Design (v4):
- bf16 on the wire and on-chip (host casts inputs); PSUM stays fp32.
- inputs land part-major as 2 large DMAs per tensor, issued on BOTH
  hardware DGE queues (sync + scalar) so transfers overlap; weights
  are packed into one tensor. v3 serialized 35 issues on one queue and
  the first matmul waited 20us.
- attention loop (query-half, head, key-tile): one kT weight load
  feeds 2 score MMs, one 1024-wide ACTIVATE per key tile, PV
  accumulates into a 2-bank PSUM tile.
- the k/v projections, v-transposes, and Wo(qh0) are INTERLEAVED into
  the attention kt loops (deadline-driven), so the PE stream stays
  dense — v3 ran 67% of the kernel HAM-throttled at 1.2 GHz because
  the ACT-bound attention loop left periodic PE idle gaps.
- masking is baked into v_aug: columns 64:128 are ones (PV emits the
  softmax denominator on partitions 64:128) and masked key rows are
  zeroed via a keep-vector input, so masked keys drop out of both
  numerator and denominator — no exp bias anywhere. exp(junk) is
  finite and multiplied by zero.
- normalize: DVE copy of denominator rows to SBUF, then
  reciprocal_approx_fast SBUF->SBUF at partition base 0 (rafast
  directly on PSUM at base 64 returned garbage on HW), then one
  tensor_mul.
- all PSUM work shares one rotating 2-bank tag (sc) + a 2-bank pv
  tag: 8 banks exactly, both double-buffered.
"""

import math
import os

import ml_dtypes
import numpy as np

from concourse import bacc
import concourse.mybir as mybir
import concourse.tile as tile
from concourse.bass_utils import run_bass_kernel_spmd
from concourse.masks import make_identity

F32 = mybir.dt.float32
BF16 = mybir.dt.bfloat16
EXP = mybir.ActivationFunctionType.Exp

B, S, D, HID, H, DH = 2, 2048, 512, 512, 8, 64


def _build(nkt_e: int, nkt_o: int):
    """One BIR program, same on all 8 cores. nkt_e/nkt_o = number of
    128-key tiles for the even/odd head (from vl[0]/vl[1])."""
    nc = bacc.Bacc("TRN2", target_bir_lowering=False, debug=False,
                   num_devices=8)
    NKT = (nkt_e, nkt_o)
    NKTM = max(NKT)
    KMAX = NKTM * 128
    NCH = (KMAX + 511) // 512          # k/v projection chunks
    KH = min(1024, KMAX)               # first-half split for k/v DMAs
    LONG = 0 if nkt_e >= nkt_o else 1  # head with more key tiles

    qT_d = nc.dram_tensor("qT", [128, 4, S], BF16, kind="ExternalInput").ap()
    kT_d = nc.dram_tensor("kT", [128, 4, KMAX], BF16, kind="ExternalInput").ap()
    vT_d = nc.dram_tensor("vT", [128, 4, KMAX], BF16, kind="ExternalInput").ap()
    wqkv_d = nc.dram_tensor("wqkv", [128, 12, 128], BF16,
                            kind="ExternalInput").ap()
    wo_d = nc.dram_tensor("wo", [128, 512], BF16, kind="ExternalInput").ap()
    keep_d = [nc.dram_tensor(f"keep{l}", [128, 1], F32,
                             kind="ExternalInput").ap() for l in range(2)]
    out_d = nc.dram_tensor("out", [16, 128, 512], BF16,
                           kind="ExternalOutput").ap()

    with tile.TileContext(nc) as tc:
        with (
            tc.tile_pool(name="consts", bufs=1) as consts,
            tc.tile_pool(name="inputs", bufs=1) as inputs,
            tc.tile_pool(name="work", bufs=1) as work,
            tc.tile_pool(name="exps", bufs=3) as exps,
            tc.tile_pool(name="recp", bufs=2) as recp,
            tc.tile_pool(name="sop", bufs=3) as sop,
            tc.tile_pool(name="psc", bufs=2, space="PSUM") as psc,
            tc.tile_pool(name="ppv", bufs=2, space="PSUM") as ppv,
        ):
            ident = consts.tile([128, 128], F32)
            make_identity(nc, ident[:])
            wqkv_s = consts.tile([128, 12, 128], BF16)
            wo_s = consts.tile([128, 512], BF16)
            keep_s = []
            # scalar-queue DMAs: weights first, then kT halves
            nc.scalar.dma_start(wqkv_s[:], wqkv_d[:])
            for l in range(2):
                m = consts.tile([128, 1], F32, tag=f"keep{l}")
                nc.scalar.dma_start(m[:], keep_d[l][:])
                keep_s.append(m)
            nc.scalar.dma_start(wo_s[:], wo_d[:])

            qT_in = inputs.tile([128, 4, S], BF16)
            kT_in = inputs.tile([128, 4, KMAX], BF16)
            vT_in = inputs.tile([128, 4, KMAX], BF16)
            nc.sync.dma_start(qT_in[:, :, 0:1024], qT_d[:, :, 0:1024])
            nc.scalar.dma_start(kT_in[:, :, 0:KH], kT_d[:, :, 0:KH])
            nc.sync.dma_start(vT_in[:, :, 0:KH], vT_d[:, :, 0:KH])
            nc.sync.dma_start(qT_in[:, :, 1024:S], qT_d[:, :, 1024:S])
            if KMAX > KH:
                nc.scalar.dma_start(kT_in[:, :, KH:KMAX], kT_d[:, :, KH:KMAX])
                nc.sync.dma_start(vT_in[:, :, KH:KMAX], vT_d[:, :, KH:KMAX])

            qTp = work.tile([128, S], BF16)      # [2*64 head rows, q]
            kTp = work.tile([128, KMAX], BF16)   # rows l*64.., keys
            vTp = work.tile([128, KMAX], F32)
            vaug = work.tile([128, NKTM, 2, 128], BF16)
            outT = work.tile([128, S], BF16)
            nc.vector.memset(vaug[:, :, :, 64:128], 1.0)

            def mix_tile(name):
                return psc.tile([128, 1024], F32, tag="sc", name=name)

            cp_s = nc.scalar.copy               # ACT copy (prologue)
            cp_v = nc.vector.tensor_copy        # DVE copy (interleaved)

            def emit_qproj(c, cp):
                ps = mix_tile("psq")
                for dt in range(4):
                    nc.tensor.matmul(ps[:, 0:512], wqkv_s[:, dt],
                                     qT_in[:, dt, c * 512:(c + 1) * 512],
                                     start=(dt == 0), stop=(dt == 3))
                cp(qTp[:, c * 512:(c + 1) * 512], ps[:, 0:512])

            def emit_kvproj(which, c, cp):
                pos = c * 512
                ncols = min(512, KMAX - pos)
                ps = mix_tile("pskv")
                o = ps[:, 0:ncols]
                for dt in range(4):
                    nc.tensor.matmul(o, wqkv_s[:, 4 * (1 + which) + dt],
                                     (kT_in if which == 0 else vT_in)
                                     [:, dt, pos:pos + ncols],
                                     start=(dt == 0), stop=(dt == 3))
                cp((kTp if which == 0 else vTp)[:, pos:pos + ncols], o)

            def emit_tp(kt):
                ps = mix_tile("ptp")
                nc.tensor.transpose(ps[:, 0:128],
                                    vTp[:, kt * 128:(kt + 1) * 128], ident[:])
                nc.vector.tensor_copy(
                    vaug[:, kt, :, 0:64],
                    ps[:, 0:128].rearrange("p (h d) -> p h d", h=2))

            def emit_keepmul(l):
                nc.gpsimd.tensor_scalar_mul(
                    vaug[:, NKT[l] - 1, l, :], vaug[:, NKT[l] - 1, l, :],
                    keep_s[l][:])

            def emit_wo(qt):
                ps = mix_tile("po")
                nc.tensor.matmul(ps[:, 0:512], outT[:, qt * 128:(qt + 1) * 128],
                                 wo_s[:], start=True, stop=True)
                so = sop.tile([128, 512], BF16, tag="so", name="so")
                nc.vector.tensor_copy(so[:], ps[:, 0:512])
                nc.sync.dma_start(out_d[qt], so[:])

            # ---- prologue: q proj, first k/v chunks, first transposes ----
            NPRO = min(2, NCH)           # chunks covered by the first halves
            for c in range(2):
                emit_qproj(c, cp_s)
            for c in range(NPRO):
                emit_kvproj(0, c, cp_s)
            for c in range(NPRO):
                emit_kvproj(1, c, cp_s)
            for kt in range(min(4 * NPRO, NKTM)):
                emit_tp(kt)
            for l in range(2):
                if NKT[l] - 1 < 4 * NPRO:
                    emit_keepmul(l)

            # deadline-tagged pending work, interleaved into qh0's long head
            pend = [(lambda c=c: emit_qproj(c, cp_v), 10 ** 6)
                    for c in range(2, 4)]
            for c in range(NPRO, NCH):
                pend.append((lambda c=c: emit_kvproj(0, c, cp_v), 4 * c))
                pend.append((lambda c=c: emit_kvproj(1, c, cp_v), 4 * c))
                for kt in range(4 * c, min(4 * c + 4, NKTM)):
                    pend.append((lambda kt=kt: emit_tp(kt), kt))
                    for l in range(2):
                        if NKT[l] - 1 == kt:
                            pend.append((lambda l=l: emit_keepmul(l), kt))

            def drain_pend(i):
                # emit everything due before attention kt i+1, plus one
                while pend and pend[0][1] <= i + 1:
                    pend.pop(0)[0]()
                if pend:
                    pend.pop(0)[0]()

            wo_pend = []

            def attention(qh, l, interleave):
                nkt = NKT[l]
                pv = ppv.tile([128, 1024], F32, tag="pv", name="pv")
                for kt in range(nkt):
                    sc = psc.tile([128, 1024], F32, tag="sc", name="sc")
                    for j in range(2):
                        qc = qh * 2 + j
                        nc.tensor.matmul(
                            sc[:, j * 512:(j + 1) * 512],
                            kTp[l * 64:(l + 1) * 64, kt * 128:(kt + 1) * 128],
                            qTp[l * 64:(l + 1) * 64, qc * 512:(qc + 1) * 512],
                            start=True, stop=True)
                    es = exps.tile([128, 1024], BF16, tag="es", name="es")
                    nc.scalar.activation(es[:], sc[:], EXP, scale=0.125)
                    for j in range(2):
                        nc.tensor.matmul(
                            pv[:, j * 512:(j + 1) * 512], vaug[:, kt, l, :],
                            es[:, j * 512:(j + 1) * 512],
                            start=(kt == 0), stop=(kt == nkt - 1))
                    if interleave == "pend":
                        drain_pend(kt)
                    elif interleave == "wo" and wo_pend:
                        emit_wo(wo_pend.pop(0))
                dens = recp.tile([64, 1024], F32, tag="dens", name="dens")
                nc.vector.tensor_copy(dens[:], pv[64:128, :])
                rec = recp.tile([64, 1024], F32, tag="rec", name="rec")
                nc.vector.reciprocal_approx_fast(rec[:], dens[:])
                nc.vector.tensor_mul(
                    outT[l * 64:(l + 1) * 64, qh * 1024:(qh + 1) * 1024],
                    pv[0:64, :], rec[:])

            # ---- attention + Wo ----
            attention(0, LONG, "pend")
            while pend:
                pend.pop(0)[0]()
            attention(0, 1 - LONG, None)
            wo_pend = list(range(8))
            attention(1, LONG, "wo")
            while wo_pend:
                emit_wo(wo_pend.pop(0))
            attention(1, 1 - LONG, None)
            for qt in range(8, 16):
                emit_wo(qt)
    nc.compile()
    return nc


_CACHE: dict = {}


def kernel(query, key, value, Wq, Wk, Wv, Wo, valid_length):
    query = np.asarray(query, np.float32)
    key = np.asarray(key, np.float32)
    value = np.asarray(value, np.float32)
    Wq = np.asarray(Wq, np.float32); Wk = np.asarray(Wk, np.float32)
    Wv = np.asarray(Wv, np.float32); Wo = np.asarray(Wo, np.float32)
    vl = np.asarray(valid_length).astype(np.int64)
    # head h is masked with vl[h % 2] (reference's np.tile quirk)
    nkt = [max(1, int(math.ceil(int(vl[l]) / 128))) for l in range(2)]

    key_ = (nkt[0], nkt[1])
    if key_ not in _CACHE:
        _CACHE[key_] = _build(*key_)
    nc = _CACHE[key_]
    KMAX = max(nkt) * 128

    bf = lambda a: np.ascontiguousarray(a.astype(ml_dtypes.bfloat16))
    pm = lambda a, n: np.ascontiguousarray(         # [n*128, N] -> [128, n, N]
        a.reshape(n, 128, -1).transpose(1, 0, 2))
    keeps = []
    for l in range(2):
        base = (nkt[l] - 1) * 128
        m = (base + np.arange(128) < int(vl[l])).astype(np.float32)
        keeps.append(np.ascontiguousarray(m.reshape(128, 1)))

    in_maps = []
    for c in range(8):
        b, p = c // 4, c % 4
        wqkv = np.concatenate(
            [pm(bf(W[:, p * 128:(p + 1) * 128]), 4) for W in (Wq, Wk, Wv)],
            axis=1)
        im = {
            "qT": pm(bf(query[b].T), 4),
            "kT": pm(bf(key[b, :KMAX].T), 4),
            "vT": pm(bf(value[b, :KMAX].T), 4),
            "wqkv": np.ascontiguousarray(wqkv),
            "wo": bf(Wo[p * 128:(p + 1) * 128]),
            "keep0": keeps[0], "keep1": keeps[1],
        }
        in_maps.append(im)

    trace = os.environ.get("BASS_KTRACE", "0") == "1"
    kw = dict(trace=True, trace_cores=list(range(8))) if trace else {}
    res = run_bass_kernel_spmd(nc, in_maps, core_ids=list(range(8)), **kw)
    kernel.last_results = res
    out = np.zeros((B, S, HID), np.float32)
    for c in range(8):
        b = c // 4
        r = np.asarray(res.results[c]["out"], dtype=np.float32)
        out[b] += r.reshape(S, HID)
    return out


# revision 8
# speedup vs baseline: 2.5591x; 1.1800x over previous
"""Trainium2 Bass kernel for masked multi-head attention (8-core SPMD).

Problem: B=2, S=2048, d_in=hid=512, H=8 heads (dh=64), fp32 in/out.
Reference quirk: the mask uses np.tile(valid_length, H), so scores row
i = b*H + h is masked with valid_length[h % 2] — head PARITY, not batch.

Sharding (8 cores): core c = (batch b = c//4, head-pair p = c%4).
Each core computes heads {2p, 2p+1} of batch b over all 2048 queries,
producing a partial [2048, 512] through its 128 rows of Wo; the host
sums the 4 pair-partials per batch.

Design (v4):
- bf16 on the wire and on-chip (host casts inputs); PSUM stays fp32.
- inputs land part-major as 2 large DMAs per tensor, issued on BOTH
  hardware DGE queues (sync + scalar) so transfers overlap; weights
  are packed into one tensor. v3 serialized 35 issues on one queue and
  the first matmul waited 20us.
- attention loop (query-half, head, key-tile): one kT weight load
  feeds 2 score MMs, one 1024-wide ACTIVATE per key tile, PV
  accumulates into a 2-bank PSUM tile.
- the k/v projections, v-transposes, and Wo(qh0) are INTERLEAVED into
  the attention kt loops (deadline-driven), so the PE stream stays
  dense — v3 ran 67% of the kernel HAM-throttled at 1.2 GHz because
  the ACT-bound attention loop left periodic PE idle gaps.
- masking is baked into v_aug: columns 64:128 are ones (PV emits the
  softmax denominator on partitions 64:128) and masked key rows are
  zeroed via a keep-vector input, so masked keys drop out of both
  numerator and denominator — no exp bias anywhere. exp(junk) is
  finite and multiplied by zero.
- normalize: DVE copy of denominator rows to SBUF, then
  reciprocal_approx_fast SBUF->SBUF at partition base 0 (rafast
  directly on PSUM at base 64 returned garbage on HW), then one
  tensor_mul.
- all PSUM work shares one rotating 2-bank tag (sc) + a 2-bank pv
  tag: 8 banks exactly, both double-buffered.
"""

import math
import os

import ml_dtypes
import numpy as np

from concourse import bacc
import concourse.mybir as mybir
import concourse.tile as tile
from concourse.bass_utils import run_bass_kernel_spmd
from concourse.masks import make_identity

F32 = mybir.dt.float32
BF16 = mybir.dt.bfloat16
EXP = mybir.ActivationFunctionType.Exp

B, S, D, HID, H, DH = 2, 2048, 512, 512, 8, 64


def _build(nkt_e: int, nkt_o: int):
    """One BIR program, same on all 8 cores. nkt_e/nkt_o = number of
    128-key tiles for the even/odd head (from vl[0]/vl[1])."""
    nc = bacc.Bacc("TRN2", target_bir_lowering=False, debug=False,
                   num_devices=8)
    NKT = (nkt_e, nkt_o)
    NKTM = max(NKT)
    KMAX = NKTM * 128
    NCH = (KMAX + 511) // 512          # k/v projection chunks
    KH = min(1024, KMAX)               # first-half split for k/v DMAs
    LONG = 0 if nkt_e >= nkt_o else 1  # head with more key tiles

    qT_d = nc.dram_tensor("qT", [128, 4, S], BF16, kind="ExternalInput").ap()
    kT_d = nc.dram_tensor("kT", [128, 4, KMAX], BF16, kind="ExternalInput").ap()
    vT_d = nc.dram_tensor("vT", [128, 4, KMAX], BF16, kind="ExternalInput").ap()
    wqkv_d = nc.dram_tensor("wqkv", [128, 12, 128], BF16,
                            kind="ExternalInput").ap()
    wo_d = nc.dram_tensor("wo", [128, 512], BF16, kind="ExternalInput").ap()
    keep_d = [nc.dram_tensor(f"keep{l}", [128, 1], F32,
                             kind="ExternalInput").ap() for l in range(2)]
    out_d = nc.dram_tensor("out", [16, 128, 512], BF16,
                           kind="ExternalOutput").ap()

    with tile.TileContext(nc) as tc:
        with (
            tc.tile_pool(name="consts", bufs=1) as consts,
            tc.tile_pool(name="inputs", bufs=1) as inputs,
            tc.tile_pool(name="work", bufs=1) as work,
            tc.tile_pool(name="exps", bufs=4) as exps,
            tc.tile_pool(name="recp", bufs=2) as recp,
            tc.tile_pool(name="sop", bufs=3) as sop,
            tc.tile_pool(name="psc", bufs=2, space="PSUM") as psc,
            tc.tile_pool(name="ppv", bufs=2, space="PSUM") as ppv,
        ):
            ident = consts.tile([128, 128], F32)
            make_identity(nc, ident[:])
            wqkv_s = consts.tile([128, 12, 128], BF16)
            wo_s = consts.tile([128, 512], BF16)
            keep_s = []
            # scalar-queue DMAs: weights first, then kT halves
            nc.scalar.dma_start(wqkv_s[:], wqkv_d[:])
            for l in range(2):
                m = consts.tile([128, 1], F32, tag=f"keep{l}")
                nc.scalar.dma_start(m[:], keep_d[l][:])
                keep_s.append(m)
            nc.scalar.dma_start(wo_s[:], wo_d[:])

            qT_in = inputs.tile([128, 4, S], BF16)
            kT_in = inputs.tile([128, 4, KMAX], BF16)
            vT_in = inputs.tile([128, 4, KMAX], BF16)
            nc.sync.dma_start(qT_in[:, :, 0:1024], qT_d[:, :, 0:1024])
            nc.scalar.dma_start(kT_in[:, :, 0:KH], kT_d[:, :, 0:KH])
            nc.sync.dma_start(vT_in[:, :, 0:KH], vT_d[:, :, 0:KH])
            if KMAX > KH:
                nc.scalar.dma_start(kT_in[:, :, KH:KMAX], kT_d[:, :, KH:KMAX])
                nc.sync.dma_start(vT_in[:, :, KH:KMAX], vT_d[:, :, KH:KMAX])
            nc.sync.dma_start(qT_in[:, :, 1024:S], qT_d[:, :, 1024:S])

            qTp = work.tile([128, S], BF16)      # [2*64 head rows, q]
            kTp = work.tile([128, KMAX], BF16)   # rows l*64.., keys
            vTp = work.tile([128, KMAX], F32)
            vaug = work.tile([128, NKTM, 2, 128], BF16)
            outT = work.tile([128, S], BF16)
            nc.vector.memset(vaug[:, :, :, 64:128], 1.0)

            def mix_tile(name):
                return psc.tile([128, 1024], F32, tag="sc", name=name)

            cp_s = nc.scalar.copy               # ACT copy (prologue)
            cp_v = nc.vector.tensor_copy        # DVE copy (interleaved)

            def emit_qproj(c, cp):
                ps = mix_tile("psq")
                for dt in range(4):
                    nc.tensor.matmul(ps[:, 0:512], wqkv_s[:, dt],
                                     qT_in[:, dt, c * 512:(c + 1) * 512],
                                     start=(dt == 0), stop=(dt == 3))
                cp(qTp[:, c * 512:(c + 1) * 512], ps[:, 0:512])

            def emit_kvproj(which, c, cp):
                pos = c * 512
                ncols = min(512, KMAX - pos)
                ps = mix_tile("pskv")
                o = ps[:, 0:ncols]
                for dt in range(4):
                    nc.tensor.matmul(o, wqkv_s[:, 4 * (1 + which) + dt],
                                     (kT_in if which == 0 else vT_in)
                                     [:, dt, pos:pos + ncols],
                                     start=(dt == 0), stop=(dt == 3))
                cp((kTp if which == 0 else vTp)[:, pos:pos + ncols], o)

            def emit_tp(kt):
                ps = mix_tile("ptp")
                nc.tensor.transpose(ps[:, 0:128],
                                    vTp[:, kt * 128:(kt + 1) * 128], ident[:])
                nc.vector.tensor_copy(
                    vaug[:, kt, :, 0:64],
                    ps[:, 0:128].rearrange("p (h d) -> p h d", h=2))

            def emit_keepmul(l):
                nc.gpsimd.tensor_scalar_mul(
                    vaug[:, NKT[l] - 1, l, :], vaug[:, NKT[l] - 1, l, :],
                    keep_s[l][:])

            def emit_wo(qt, cp):
                ps = mix_tile("po")
                nc.tensor.matmul(ps[:, 0:512], outT[:, qt * 128:(qt + 1) * 128],
                                 wo_s[:], start=True, stop=True)
                so = sop.tile([128, 512], BF16, tag="so", name="so")
                cp(so[:], ps[:, 0:512])
                nc.sync.dma_start(out_d[qt], so[:])

            # ---- prologue: q proj, first k/v chunks, first transposes ----
            NPRO = min(2, NCH)           # chunks covered by the first halves
            for c in range(2):
                emit_qproj(c, cp_s)
            for c in range(NPRO):
                emit_kvproj(0, c, cp_s)
            for c in range(NPRO):
                emit_kvproj(1, c, cp_s)
            for kt in range(min(4 * NPRO, NKTM)):
                emit_tp(kt)
            for l in range(2):
                if NKT[l] - 1 < 4 * NPRO:
                    emit_keepmul(l)

            # deadline-tagged pending work, interleaved into qh0's long head
            pend = []
            for c in range(NPRO, NCH):
                pend.append((lambda c=c: emit_kvproj(0, c, cp_v), 4 * c))
                pend.append((lambda c=c: emit_kvproj(1, c, cp_v), 4 * c))
                for kt in range(4 * c, min(4 * c + 4, NKTM)):
                    pend.append((lambda kt=kt: emit_tp(kt), kt))
                    for l in range(2):
                        if NKT[l] - 1 == kt:
                            pend.append((lambda l=l: emit_keepmul(l), kt))
            pend += [(lambda c=c: emit_qproj(c, cp_v), 10 ** 6)
                     for c in range(2, 4)]

            def drain_pend(i):
                # emit everything due before attention kt i+1, plus one
                while pend and pend[0][1] <= i + 1:
                    pend.pop(0)[0]()
                if pend:
                    pend.pop(0)[0]()

            wo_pend = []

            def attention(qh, l, interleave):
                nkt = NKT[l]
                pv = ppv.tile([128, 1024], F32, tag="pv", name="pv")

                def emit_pv(kt, es):
                    for j in range(2):
                        nc.tensor.matmul(
                            pv[:, j * 512:(j + 1) * 512], vaug[:, kt, l, :],
                            es[:, j * 512:(j + 1) * 512],
                            start=(kt == 0), stop=(kt == nkt - 1))

                prev = None
                for kt in range(nkt):
                    sc = psc.tile([128, 1024], F32, tag="sc", name="sc")
                    for j in range(2):
                        qc = qh * 2 + j
                        nc.tensor.matmul(
                            sc[:, j * 512:(j + 1) * 512],
                            kTp[l * 64:(l + 1) * 64, kt * 128:(kt + 1) * 128],
                            qTp[l * 64:(l + 1) * 64, qc * 512:(qc + 1) * 512],
                            start=True, stop=True)
                    es = exps.tile([128, 1024], BF16, tag="es", name="es")
                    nc.scalar.activation(es[:], sc[:], EXP, scale=0.125)
                    # PV runs one stage behind exp so the PE never waits
                    # on the current tile's ACT
                    if prev is not None:
                        emit_pv(*prev)
                    prev = (kt, es)
                    if interleave == "pend":
                        drain_pend(kt)
                    elif interleave == "wo" and kt >= 3 and wo_pend:
                        emit_wo(wo_pend.pop(0), cp_v)
                emit_pv(*prev)
                dens = recp.tile([64, 1024], F32, tag="dens", name="dens")
                nc.vector.tensor_copy(dens[:], pv[64:128, :])
                rec = recp.tile([64, 1024], F32, tag="rec", name="rec")
                nc.vector.reciprocal_approx_fast(rec[:], dens[:])
                nc.vector.tensor_mul(
                    outT[l * 64:(l + 1) * 64, qh * 1024:(qh + 1) * 1024],
                    pv[0:64, :], rec[:])

            # ---- attention + Wo ----
            attention(0, LONG, "pend")
            while pend:
                pend.pop(0)[0]()
            attention(0, 1 - LONG, None)
            wo_pend = list(range(8))
            attention(1, LONG, "wo")
            while wo_pend:
                emit_wo(wo_pend.pop(0), cp_v)
            attention(1, 1 - LONG, None)
            for qt in range(8, 16):
                emit_wo(qt, cp_s)
    nc.compile()
    return nc


_CACHE: dict = {}


def kernel(query, key, value, Wq, Wk, Wv, Wo, valid_length):
    query = np.asarray(query, np.float32)
    key = np.asarray(key, np.float32)
    value = np.asarray(value, np.float32)
    Wq = np.asarray(Wq, np.float32); Wk = np.asarray(Wk, np.float32)
    Wv = np.asarray(Wv, np.float32); Wo = np.asarray(Wo, np.float32)
    vl = np.asarray(valid_length).astype(np.int64)
    # head h is masked with vl[h % 2] (reference's np.tile quirk)
    nkt = [max(1, int(math.ceil(int(vl[l]) / 128))) for l in range(2)]

    key_ = (nkt[0], nkt[1])
    if key_ not in _CACHE:
        _CACHE[key_] = _build(*key_)
    nc = _CACHE[key_]
    KMAX = max(nkt) * 128

    bf = lambda a: np.ascontiguousarray(a.astype(ml_dtypes.bfloat16))
    pm = lambda a, n: np.ascontiguousarray(         # [n*128, N] -> [128, n, N]
        a.reshape(n, 128, -1).transpose(1, 0, 2))
    keeps = []
    for l in range(2):
        base = (nkt[l] - 1) * 128
        m = (base + np.arange(128) < int(vl[l])).astype(np.float32)
        keeps.append(np.ascontiguousarray(m.reshape(128, 1)))

    in_maps = []
    for c in range(8):
        b, p = c // 4, c % 4
        wqkv = np.concatenate(
            [pm(bf(W[:, p * 128:(p + 1) * 128]), 4) for W in (Wq, Wk, Wv)],
            axis=1)
        im = {
            "qT": pm(bf(query[b].T), 4),
            "kT": pm(bf(key[b, :KMAX].T), 4),
            "vT": pm(bf(value[b, :KMAX].T), 4),
            "wqkv": np.ascontiguousarray(wqkv),
            "wo": bf(Wo[p * 128:(p + 1) * 128]),
            "keep0": keeps[0], "keep1": keeps[1],
        }
        in_maps.append(im)

    trace = os.environ.get("BASS_KTRACE", "0") == "1"
    kw = dict(trace=True, trace_cores=list(range(8))) if trace else {}
    res = run_bass_kernel_spmd(nc, in_maps, core_ids=list(range(8)), **kw)
    kernel.last_results = res
    out = np.zeros((B, S, HID), np.float32)
    for c in range(8):
        b = c // 4
        r = np.asarray(res.results[c]["out"], dtype=np.float32)
        out[b] += r.reshape(S, HID)
    return out


# revision 9
# speedup vs baseline: 2.6062x; 1.0184x over previous
"""Trainium2 Bass kernel for masked multi-head attention (8-core SPMD).

Problem: B=2, S=2048, d_in=hid=512, H=8 heads (dh=64), fp32 in/out.
Reference quirk: the mask uses np.tile(valid_length, H), so scores row
i = b*H + h is masked with valid_length[h % 2] — head PARITY, not batch.

Sharding (8 cores): core c = (batch b = c//4, head-pair p = c%4).
Each core computes heads {2p, 2p+1} of batch b over all 2048 queries,
producing a partial [2048, 512] through its 128 rows of Wo; the host
sums the 4 pair-partials per batch.

Design (v4):
- bf16 on the wire and on-chip (host casts inputs); PSUM stays fp32.
- inputs land part-major as 2 large DMAs per tensor, issued on BOTH
  hardware DGE queues (sync + scalar) so transfers overlap; weights
  are packed into one tensor. v3 serialized 35 issues on one queue and
  the first matmul waited 20us.
- attention loop (query-half, head, key-tile): one kT weight load
  feeds 2 score MMs, one 1024-wide ACTIVATE per key tile, PV
  accumulates into a 2-bank PSUM tile.
- the k/v projections, v-transposes, and Wo(qh0) are INTERLEAVED into
  the attention kt loops (deadline-driven), so the PE stream stays
  dense — v3 ran 67% of the kernel HAM-throttled at 1.2 GHz because
  the ACT-bound attention loop left periodic PE idle gaps.
- masking is baked into v_aug: columns 64:128 are ones (PV emits the
  softmax denominator on partitions 64:128) and masked key rows are
  zeroed via a keep-vector input, so masked keys drop out of both
  numerator and denominator — no exp bias anywhere. exp(junk) is
  finite and multiplied by zero.
- normalize: DVE copy of denominator rows to SBUF, then
  reciprocal_approx_fast SBUF->SBUF at partition base 0 (rafast
  directly on PSUM at base 64 returned garbage on HW), then one
  tensor_mul.
- all PSUM work shares one rotating 2-bank tag (sc) + a 2-bank pv
  tag: 8 banks exactly, both double-buffered.
"""

import math
import os

import ml_dtypes
import numpy as np

from concourse import bacc
import concourse.mybir as mybir
import concourse.tile as tile
from concourse.bass_utils import run_bass_kernel_spmd
from concourse.masks import make_identity

F32 = mybir.dt.float32
BF16 = mybir.dt.bfloat16
EXP = mybir.ActivationFunctionType.Exp

B, S, D, HID, H, DH = 2, 2048, 512, 512, 8, 64


def _build(nkt_e: int, nkt_o: int):
    """One BIR program, same on all 8 cores. nkt_e/nkt_o = number of
    128-key tiles for the even/odd head (from vl[0]/vl[1])."""
    nc = bacc.Bacc("TRN2", target_bir_lowering=False, debug=False,
                   num_devices=8)
    NKT = (nkt_e, nkt_o)
    NKTM = max(NKT)
    KMAX = NKTM * 128
    NCH = (KMAX + 511) // 512          # k/v projection chunks
    KH = min(1024, KMAX)               # first-half split for k/v DMAs
    LONG = 0 if nkt_e >= nkt_o else 1  # head with more key tiles

    qT_d = nc.dram_tensor("qT", [128, 4, S], BF16, kind="ExternalInput").ap()
    kT_d = nc.dram_tensor("kT", [128, 4, KMAX], BF16, kind="ExternalInput").ap()
    vT_d = nc.dram_tensor("vT", [128, 4, KMAX], BF16, kind="ExternalInput").ap()
    wqkv_d = nc.dram_tensor("wqkv", [128, 12, 128], BF16,
                            kind="ExternalInput").ap()
    wo_d = nc.dram_tensor("wo", [128, 512], BF16, kind="ExternalInput").ap()
    keep_d = [nc.dram_tensor(f"keep{l}", [128, 1], F32,
                             kind="ExternalInput").ap() for l in range(2)]
    out_d = nc.dram_tensor("out", [16, 128, 512], BF16,
                           kind="ExternalOutput").ap()

    with tile.TileContext(nc) as tc:
        with (
            tc.tile_pool(name="consts", bufs=1) as consts,
            tc.tile_pool(name="inputs", bufs=1) as inputs,
            tc.tile_pool(name="work", bufs=1) as work,
            tc.tile_pool(name="exps", bufs=4) as exps,
            tc.tile_pool(name="recp", bufs=2) as recp,
            tc.tile_pool(name="sop", bufs=3) as sop,
            tc.tile_pool(name="psc", bufs=2, space="PSUM") as psc,
            tc.tile_pool(name="ppv", bufs=2, space="PSUM") as ppv,
        ):
            ident = consts.tile([128, 128], F32)
            make_identity(nc, ident[:])
            wqkv_s = consts.tile([128, 12, 128], BF16)
            wo_s = consts.tile([128, 512], BF16)
            keep_s = []
            # scalar-queue DMAs: weights first, then kT halves
            nc.scalar.dma_start(wqkv_s[:], wqkv_d[:])
            for l in range(2):
                m = consts.tile([128, 1], F32, tag=f"keep{l}")
                nc.scalar.dma_start(m[:], keep_d[l][:])
                keep_s.append(m)
            nc.scalar.dma_start(wo_s[:], wo_d[:])

            qT_in = inputs.tile([128, 4, S], BF16)
            kT_in = inputs.tile([128, 4, KMAX], BF16)
            vT_in = inputs.tile([128, 4, KMAX], BF16)
            nc.sync.dma_start(qT_in[:, :, 0:1024], qT_d[:, :, 0:1024])
            nc.scalar.dma_start(kT_in[:, :, 0:KH], kT_d[:, :, 0:KH])
            nc.sync.dma_start(vT_in[:, :, 0:KH], vT_d[:, :, 0:KH])
            if KMAX > KH:
                nc.scalar.dma_start(kT_in[:, :, KH:KMAX], kT_d[:, :, KH:KMAX])
                nc.sync.dma_start(vT_in[:, :, KH:KMAX], vT_d[:, :, KH:KMAX])
            nc.sync.dma_start(qT_in[:, :, 1024:S], qT_d[:, :, 1024:S])

            qTp = work.tile([128, S], BF16)      # [2*64 head rows, q]
            kTp = work.tile([128, KMAX], BF16)   # rows l*64.., keys
            vTp = work.tile([128, KMAX], F32)
            vaug = work.tile([128, NKTM, 2, 128], BF16)
            outT = work.tile([128, S], BF16)
            nc.vector.memset(vaug[:, :, :, 64:128], 1.0)

            def mix_tile(name):
                return psc.tile([128, 1024], F32, tag="sc", name=name)

            cp_s = nc.scalar.copy               # ACT copy (prologue)
            cp_v = nc.vector.tensor_copy        # DVE copy (interleaved)

            def emit_qproj(c, cp):
                ps = mix_tile("psq")
                for dt in range(4):
                    nc.tensor.matmul(ps[:, 0:512], wqkv_s[:, dt],
                                     qT_in[:, dt, c * 512:(c + 1) * 512],
                                     start=(dt == 0), stop=(dt == 3))
                cp(qTp[:, c * 512:(c + 1) * 512], ps[:, 0:512])

            def emit_kvproj(which, c, cp):
                pos = c * 512
                ncols = min(512, KMAX - pos)
                ps = mix_tile("pskv")
                o = ps[:, 0:ncols]
                for dt in range(4):
                    nc.tensor.matmul(o, wqkv_s[:, 4 * (1 + which) + dt],
                                     (kT_in if which == 0 else vT_in)
                                     [:, dt, pos:pos + ncols],
                                     start=(dt == 0), stop=(dt == 3))
                cp((kTp if which == 0 else vTp)[:, pos:pos + ncols], o)

            def emit_tp(kt):
                ps = mix_tile("ptp")
                nc.tensor.transpose(ps[:, 0:128],
                                    vTp[:, kt * 128:(kt + 1) * 128], ident[:])
                nc.vector.tensor_copy(
                    vaug[:, kt, :, 0:64],
                    ps[:, 0:128].rearrange("p (h d) -> p h d", h=2))

            def emit_keepmul(l):
                nc.gpsimd.tensor_scalar_mul(
                    vaug[:, NKT[l] - 1, l, :], vaug[:, NKT[l] - 1, l, :],
                    keep_s[l][:])

            def emit_wo(qt, cp):
                ps = mix_tile("po")
                nc.tensor.matmul(ps[:, 0:512], outT[:, qt * 128:(qt + 1) * 128],
                                 wo_s[:], start=True, stop=True)
                so = sop.tile([128, 512], BF16, tag="so", name="so")
                cp(so[:], ps[:, 0:512])
                nc.sync.dma_start(out_d[qt], so[:])

            # ---- prologue: q proj, first k/v chunks, first transposes ----
            NPRO = min(2, NCH)           # chunks covered by the first halves
            for c in range(2):
                emit_qproj(c, cp_s)
            for c in range(NPRO):
                emit_kvproj(0, c, cp_s)
            for c in range(NPRO):
                emit_kvproj(1, c, cp_s)
            for kt in range(min(4 * NPRO, NKTM)):
                emit_tp(kt)
            for l in range(2):
                if NKT[l] - 1 < 4 * NPRO:
                    emit_keepmul(l)

            # deadline-tagged pending work, interleaved into qh0's long head
            pend = []
            for c in range(NPRO, NCH):
                pend.append((lambda c=c: emit_kvproj(0, c, cp_v), 4 * c))
                pend.append((lambda c=c: emit_kvproj(1, c, cp_v), 4 * c))
                for kt in range(4 * c, min(4 * c + 4, NKTM)):
                    pend.append((lambda kt=kt: emit_tp(kt), kt))
                    for l in range(2):
                        if NKT[l] - 1 == kt:
                            pend.append((lambda l=l: emit_keepmul(l), kt))
            pend += [(lambda c=c: emit_qproj(c, cp_v), 10 ** 6)
                     for c in range(2, 4)]

            def drain_pend(i):
                # emit everything due before attention kt i+1, plus one
                while pend and pend[0][1] <= i + 1:
                    pend.pop(0)[0]()
                if pend:
                    pend.pop(0)[0]()

            wo_pend = []

            def attention(qh, l, interleave):
                nkt = NKT[l]
                pv = ppv.tile([128, 1024], F32, tag="pv", name="pv")

                def emit_pv(kt, es):
                    for j in range(2):
                        nc.tensor.matmul(
                            pv[:, j * 512:(j + 1) * 512], vaug[:, kt, l, :],
                            es[:, j * 512:(j + 1) * 512],
                            start=(kt == 0), stop=(kt == nkt - 1))

                prev = None
                for kt in range(nkt):
                    sc = psc.tile([128, 1024], F32, tag="sc", name="sc")
                    for j in range(2):
                        qc = qh * 2 + j
                        nc.tensor.matmul(
                            sc[:, j * 512:(j + 1) * 512],
                            kTp[l * 64:(l + 1) * 64, kt * 128:(kt + 1) * 128],
                            qTp[l * 64:(l + 1) * 64, qc * 512:(qc + 1) * 512],
                            start=True, stop=True)
                    es = exps.tile([128, 1024], BF16, tag="es", name="es")
                    nc.scalar.activation(es[:], sc[:], EXP, scale=0.125)
                    # PV runs one stage behind exp so the PE never waits
                    # on the current tile's ACT
                    if prev is not None:
                        emit_pv(*prev)
                    prev = (kt, es)
                    if interleave == "pend":
                        drain_pend(kt)
                    elif interleave == "wo" and kt >= 3 and wo_pend:
                        emit_wo(wo_pend.pop(0), cp_v)
                emit_pv(*prev)
                dens = recp.tile([64, 1024], F32, tag="dens", name="dens")
                nc.vector.tensor_copy(dens[:], pv[64:128, :])
                rec = recp.tile([64, 1024], F32, tag="rec", name="rec")
                nc.vector.reciprocal_approx_fast(rec[:], dens[:])
                nc.vector.tensor_mul(
                    outT[l * 64:(l + 1) * 64, qh * 1024:(qh + 1) * 1024],
                    pv[0:64, :], rec[:])

            # ---- attention + Wo ----
            # short head first: phase seams are then LONG->LONG, so the
            # PE pipeline never drains on a 1-tile head mid-kernel (a
            # drained seam re-throttles the PE clock for the next phase).
            # Only valid when the prologue covered the short head's data.
            if NKT[1 - LONG] <= 4 * NPRO:
                attention(0, 1 - LONG, None)
                attention(0, LONG, "pend")
                while pend:
                    pend.pop(0)[0]()
                wo_pend = list(range(8))
                attention(1, 1 - LONG, None)
                attention(1, LONG, "wo")
                while wo_pend:
                    emit_wo(wo_pend.pop(0), cp_v)
            else:
                attention(0, LONG, "pend")
                while pend:
                    pend.pop(0)[0]()
                attention(0, 1 - LONG, None)
                wo_pend = list(range(8))
                attention(1, LONG, "wo")
                while wo_pend:
                    emit_wo(wo_pend.pop(0), cp_v)
                attention(1, 1 - LONG, None)
            for qt in range(8, 16):
                emit_wo(qt, cp_s)
    nc.compile()
    return nc


_CACHE: dict = {}


def kernel(query, key, value, Wq, Wk, Wv, Wo, valid_length):
    query = np.asarray(query, np.float32)
    key = np.asarray(key, np.float32)
    value = np.asarray(value, np.float32)
    Wq = np.asarray(Wq, np.float32); Wk = np.asarray(Wk, np.float32)
    Wv = np.asarray(Wv, np.float32); Wo = np.asarray(Wo, np.float32)
    vl = np.asarray(valid_length).astype(np.int64)
    # head h is masked with vl[h % 2] (reference's np.tile quirk)
    nkt = [max(1, int(math.ceil(int(vl[l]) / 128))) for l in range(2)]

    key_ = (nkt[0], nkt[1])
    if key_ not in _CACHE:
        _CACHE[key_] = _build(*key_)
    nc = _CACHE[key_]
    KMAX = max(nkt) * 128

    bf = lambda a: np.ascontiguousarray(a.astype(ml_dtypes.bfloat16))
    pm = lambda a, n: np.ascontiguousarray(         # [n*128, N] -> [128, n, N]
        a.reshape(n, 128, -1).transpose(1, 0, 2))
    keeps = []
    for l in range(2):
        base = (nkt[l] - 1) * 128
        m = (base + np.arange(128) < int(vl[l])).astype(np.float32)
        keeps.append(np.ascontiguousarray(m.reshape(128, 1)))

    in_maps = []
    for c in range(8):
        b, p = c // 4, c % 4
        wqkv = np.concatenate(
            [pm(bf(W[:, p * 128:(p + 1) * 128]), 4) for W in (Wq, Wk, Wv)],
            axis=1)
        im = {
            "qT": pm(bf(query[b].T), 4),
            "kT": pm(bf(key[b, :KMAX].T), 4),
            "vT": pm(bf(value[b, :KMAX].T), 4),
            "wqkv": np.ascontiguousarray(wqkv),
            "wo": bf(Wo[p * 128:(p + 1) * 128]),
            "keep0": keeps[0], "keep1": keeps[1],
        }
        in_maps.append(im)

    trace = os.environ.get("BASS_KTRACE", "0") == "1"
    kw = dict(trace=True, trace_cores=list(range(8))) if trace else {}
    res = run_bass_kernel_spmd(nc, in_maps, core_ids=list(range(8)), **kw)
    kernel.last_results = res
    out = np.zeros((B, S, HID), np.float32)
    for c in range(8):
        b = c // 4
        r = np.asarray(res.results[c]["out"], dtype=np.float32)
        out[b] += r.reshape(S, HID)
    return out


# revision 11
# speedup vs baseline: 2.6196x; 1.0051x over previous
"""Trainium2 Bass kernel for masked multi-head attention (8-core SPMD).

Problem: B=2, S=2048, d_in=hid=512, H=8 heads (dh=64), fp32 in/out.
Reference quirk: the mask uses np.tile(valid_length, H), so scores row
i = b*H + h is masked with valid_length[h % 2] — head PARITY, not batch.

Sharding (8 cores): core c = (batch b = c//4, head-pair p = c%4).
Each core computes heads {2p, 2p+1} of batch b over all 2048 queries,
producing a partial [2048, 512] through its 128 rows of Wo; the host
sums the 4 pair-partials per batch.

Design (v4):
- bf16 on the wire and on-chip (host casts inputs); PSUM stays fp32.
- inputs land part-major as 2 large DMAs per tensor, issued on BOTH
  hardware DGE queues (sync + scalar) so transfers overlap; weights
  are packed into one tensor. v3 serialized 35 issues on one queue and
  the first matmul waited 20us.
- attention loop (query-half, head, key-tile): one kT weight load
  feeds 2 score MMs, one 1024-wide ACTIVATE per key tile, PV
  accumulates into a 2-bank PSUM tile.
- the k/v projections, v-transposes, and Wo(qh0) are INTERLEAVED into
  the attention kt loops (deadline-driven), so the PE stream stays
  dense — v3 ran 67% of the kernel HAM-throttled at 1.2 GHz because
  the ACT-bound attention loop left periodic PE idle gaps.
- masking is baked into v_aug: columns 64:128 are ones (PV emits the
  softmax denominator on partitions 64:128) and masked key rows are
  zeroed via a keep-vector input, so masked keys drop out of both
  numerator and denominator — no exp bias anywhere. exp(junk) is
  finite and multiplied by zero.
- normalize: DVE copy of denominator rows to SBUF, then
  reciprocal_approx_fast SBUF->SBUF at partition base 0 (rafast
  directly on PSUM at base 64 returned garbage on HW), then one
  tensor_mul.
- all PSUM work shares one rotating 2-bank tag (sc) + a 2-bank pv
  tag: 8 banks exactly, both double-buffered.
"""

import math
import os

import ml_dtypes
import numpy as np

from concourse import bacc
import concourse.mybir as mybir
import concourse.tile as tile
from concourse.bass_utils import run_bass_kernel_spmd
from concourse.masks import make_identity

F32 = mybir.dt.float32
BF16 = mybir.dt.bfloat16
EXP = mybir.ActivationFunctionType.Exp

B, S, D, HID, H, DH = 2, 2048, 512, 512, 8, 64


def _build(nkt_e: int, nkt_o: int):
    """One BIR program, same on all 8 cores. nkt_e/nkt_o = number of
    128-key tiles for the even/odd head (from vl[0]/vl[1])."""
    nc = bacc.Bacc("TRN2", target_bir_lowering=False, debug=False,
                   num_devices=8)
    NKT = (nkt_e, nkt_o)
    NKTM = max(NKT)
    KMAX = NKTM * 128
    NCH = (KMAX + 511) // 512          # k/v projection chunks
    KH = min(1024, KMAX)               # first-half split for k/v DMAs
    LONG = 0 if nkt_e >= nkt_o else 1  # head with more key tiles

    qT_d = nc.dram_tensor("qT", [128, 4, S], BF16, kind="ExternalInput").ap()
    kT_d = nc.dram_tensor("kT", [128, 4, KMAX], BF16, kind="ExternalInput").ap()
    vT_d = nc.dram_tensor("vT", [128, 4, KMAX], BF16, kind="ExternalInput").ap()
    wqkv_d = nc.dram_tensor("wqkv", [128, 12, 128], BF16,
                            kind="ExternalInput").ap()
    wo_d = nc.dram_tensor("wo", [128, 512], BF16, kind="ExternalInput").ap()
    keep_d = [nc.dram_tensor(f"keep{l}", [128, 1], F32,
                             kind="ExternalInput").ap() for l in range(2)]
    out_d = nc.dram_tensor("out", [16, 128, 512], BF16,
                           kind="ExternalOutput").ap()

    with tile.TileContext(nc) as tc:
        with (
            tc.tile_pool(name="consts", bufs=1) as consts,
            tc.tile_pool(name="inputs", bufs=1) as inputs,
            tc.tile_pool(name="work", bufs=1) as work,
            tc.tile_pool(name="exps", bufs=4) as exps,
            tc.tile_pool(name="recp", bufs=2) as recp,
            tc.tile_pool(name="sop", bufs=3) as sop,
            tc.tile_pool(name="psc", bufs=2, space="PSUM") as psc,
            tc.tile_pool(name="ppv", bufs=2, space="PSUM") as ppv,
        ):
            ident = consts.tile([128, 128], F32)
            make_identity(nc, ident[:])
            wqkv_s = consts.tile([128, 12, 128], BF16)
            wo_s = consts.tile([128, 512], BF16)
            keep_s = []
            # scalar-queue DMAs: weights first, then kT halves
            nc.scalar.dma_start(wqkv_s[:], wqkv_d[:])
            for l in range(2):
                m = consts.tile([128, 1], F32, tag=f"keep{l}")
                nc.scalar.dma_start(m[:], keep_d[l][:])
                keep_s.append(m)
            nc.scalar.dma_start(wo_s[:], wo_d[:])

            qT_in = inputs.tile([128, 4, S], BF16)
            kT_in = inputs.tile([128, 4, KMAX], BF16)
            vT_in = inputs.tile([128, 4, KMAX], BF16)
            nc.sync.dma_start(qT_in[:, :, 0:1024], qT_d[:, :, 0:1024])
            nc.scalar.dma_start(kT_in[:, :, 0:KH], kT_d[:, :, 0:KH])
            nc.sync.dma_start(vT_in[:, :, 0:KH], vT_d[:, :, 0:KH])
            if KMAX > KH:
                nc.scalar.dma_start(kT_in[:, :, KH:KMAX], kT_d[:, :, KH:KMAX])
                nc.scalar.dma_start(vT_in[:, :, KH:KMAX], vT_d[:, :, KH:KMAX])
            nc.sync.dma_start(qT_in[:, :, 1024:S], qT_d[:, :, 1024:S])

            qTp = work.tile([128, S], BF16)      # [2*64 head rows, q]
            kTp = work.tile([128, KMAX], BF16)   # rows l*64.., keys
            vTp = work.tile([128, KMAX], F32)
            vaug = work.tile([128, NKTM, 2, 128], BF16)
            outT = work.tile([128, S], BF16)
            nc.vector.memset(vaug[:, :, :, 64:128], 1.0)

            def mix_tile(name):
                return psc.tile([128, 1024], F32, tag="sc", name=name)

            cp_s = nc.scalar.copy               # ACT copy (prologue)
            cp_v = nc.vector.tensor_copy        # DVE copy (interleaved)

            def emit_qproj(c, cp):
                ps = mix_tile("psq")
                for dt in range(4):
                    nc.tensor.matmul(ps[:, 0:512], wqkv_s[:, dt],
                                     qT_in[:, dt, c * 512:(c + 1) * 512],
                                     start=(dt == 0), stop=(dt == 3))
                cp(qTp[:, c * 512:(c + 1) * 512], ps[:, 0:512])

            def emit_kvproj(which, c, cp):
                pos = c * 512
                ncols = min(512, KMAX - pos)
                ps = mix_tile("pskv")
                o = ps[:, 0:ncols]
                for dt in range(4):
                    nc.tensor.matmul(o, wqkv_s[:, 4 * (1 + which) + dt],
                                     (kT_in if which == 0 else vT_in)
                                     [:, dt, pos:pos + ncols],
                                     start=(dt == 0), stop=(dt == 3))
                cp((kTp if which == 0 else vTp)[:, pos:pos + ncols], o)

            def emit_tp(kt):
                ps = mix_tile("ptp")
                nc.tensor.transpose(ps[:, 0:128],
                                    vTp[:, kt * 128:(kt + 1) * 128], ident[:])
                nc.vector.tensor_copy(
                    vaug[:, kt, :, 0:64],
                    ps[:, 0:128].rearrange("p (h d) -> p h d", h=2))

            def emit_keepmul(l):
                nc.gpsimd.tensor_scalar_mul(
                    vaug[:, NKT[l] - 1, l, :], vaug[:, NKT[l] - 1, l, :],
                    keep_s[l][:])

            def emit_wo(qt, cp):
                ps = mix_tile("po")
                nc.tensor.matmul(ps[:, 0:512], outT[:, qt * 128:(qt + 1) * 128],
                                 wo_s[:], start=True, stop=True)
                so = sop.tile([128, 512], BF16, tag="so", name="so")
                cp(so[:], ps[:, 0:512])
                nc.sync.dma_start(out_d[qt], so[:])

            # ---- prologue: q proj, first k/v chunks, first transposes ----
            NPRO = min(2, NCH)           # chunks covered by the first halves
            for c in range(2):
                emit_qproj(c, cp_s)
            for c in range(NPRO):
                emit_kvproj(0, c, cp_s)
            for c in range(NPRO):
                emit_kvproj(1, c, cp_s)
            for kt in range(min(4 * NPRO, NKTM)):
                emit_tp(kt)
            for l in range(2):
                if NKT[l] - 1 < 4 * NPRO:
                    emit_keepmul(l)

            # deadline-tagged pending work, interleaved into qh0's long head
            pend = []
            for c in range(NPRO, NCH):
                pend.append((lambda c=c: emit_kvproj(0, c, cp_v), 4 * c))
                pend.append((lambda c=c: emit_kvproj(1, c, cp_v), 4 * c))
                for kt in range(4 * c, min(4 * c + 4, NKTM)):
                    pend.append((lambda kt=kt: emit_tp(kt), kt))
                    for l in range(2):
                        if NKT[l] - 1 == kt:
                            pend.append((lambda l=l: emit_keepmul(l), kt))
            pend += [(lambda c=c: emit_qproj(c, cp_v), 10 ** 6)
                     for c in range(2, 4)]

            def drain_pend(i):
                # emit everything due before attention kt i+1, plus one
                while pend and pend[0][1] <= i + 1:
                    pend.pop(0)[0]()
                if pend:
                    pend.pop(0)[0]()

            wo_pend = []

            def attention(qh, l, interleave, pending_fin=None):
                nkt = NKT[l]
                pv = ppv.tile([128, 1024], F32, tag="pv", name="pv")

                def emit_pv(kt, es):
                    for j in range(2):
                        nc.tensor.matmul(
                            pv[:, j * 512:(j + 1) * 512], vaug[:, kt, l, :],
                            es[:, j * 512:(j + 1) * 512],
                            start=(kt == 0), stop=(kt == nkt - 1))

                prev = None
                for kt in range(nkt):
                    sc = psc.tile([128, 1024], F32, tag="sc", name="sc")
                    for j in range(2):
                        qc = qh * 2 + j
                        nc.tensor.matmul(
                            sc[:, j * 512:(j + 1) * 512],
                            kTp[l * 64:(l + 1) * 64, kt * 128:(kt + 1) * 128],
                            qTp[l * 64:(l + 1) * 64, qc * 512:(qc + 1) * 512],
                            start=True, stop=True)
                    es = exps.tile([128, 1024], BF16, tag="es", name="es")
                    nc.scalar.activation(es[:], sc[:], EXP, scale=0.125)
                    # PV runs one stage behind exp so the PE never waits
                    # on the current tile's ACT; the PREVIOUS head's final
                    # PV + normalize are deferred to our kt0 so the PE
                    # pipeline never drains at a head seam
                    if prev is not None:
                        emit_pv(*prev)
                    elif pending_fin is not None:
                        pending_fin()
                    prev = (kt, es)
                    if interleave == "pend":
                        drain_pend(kt)
                    elif interleave == "wo" and kt >= 3 and wo_pend:
                        emit_wo(wo_pend.pop(0), cp_v)

                def fin():
                    emit_pv(*prev)
                    dens = recp.tile([64, 1024], F32, tag="dens", name="dens")
                    nc.vector.tensor_copy(dens[:], pv[64:128, :])
                    rec = recp.tile([64, 1024], F32, tag="rec", name="rec")
                    nc.vector.reciprocal_approx_fast(rec[:], dens[:])
                    nc.vector.tensor_mul(
                        outT[l * 64:(l + 1) * 64, qh * 1024:(qh + 1) * 1024],
                        pv[0:64, :], rec[:])
                return fin

            # ---- attention + Wo ----
            # short head first: phase seams are then LONG->LONG, so the
            # PE pipeline never drains on a 1-tile head mid-kernel (a
            # drained seam re-throttles the PE clock for the next phase).
            # Only valid when the prologue covered the short head's data.
            if NKT[1 - LONG] <= 4 * NPRO:
                fin = attention(0, 1 - LONG, None)
                fin = attention(0, LONG, "pend", fin)
                while pend:
                    pend.pop(0)[0]()
                wo_pend = list(range(8))
                fin = attention(1, 1 - LONG, None, fin)
                fin = attention(1, LONG, "wo", fin)
                while wo_pend:
                    emit_wo(wo_pend.pop(0), cp_v)
                fin()
            else:
                fin = attention(0, LONG, "pend")
                while pend:
                    pend.pop(0)[0]()
                fin = attention(0, 1 - LONG, None, fin)
                wo_pend = list(range(8))
                fin = attention(1, LONG, "wo", fin)
                while wo_pend:
                    emit_wo(wo_pend.pop(0), cp_v)
                fin = attention(1, 1 - LONG, None, fin)
                fin()
            for qt in range(8, 16):
                emit_wo(qt, cp_s)
    nc.compile()
    return nc


_CACHE: dict = {}


def kernel(query, key, value, Wq, Wk, Wv, Wo, valid_length):
    query = np.asarray(query, np.float32)
    key = np.asarray(key, np.float32)
    value = np.asarray(value, np.float32)
    Wq = np.asarray(Wq, np.float32); Wk = np.asarray(Wk, np.float32)
    Wv = np.asarray(Wv, np.float32); Wo = np.asarray(Wo, np.float32)
    vl = np.asarray(valid_length).astype(np.int64)
    # head h is masked with vl[h % 2] (reference's np.tile quirk)
    nkt = [max(1, int(math.ceil(int(vl[l]) / 128))) for l in range(2)]

    key_ = (nkt[0], nkt[1])
    if key_ not in _CACHE:
        _CACHE[key_] = _build(*key_)
    nc = _CACHE[key_]
    KMAX = max(nkt) * 128

    bf = lambda a: np.ascontiguousarray(a.astype(ml_dtypes.bfloat16))
    pm = lambda a, n: np.ascontiguousarray(         # [n*128, N] -> [128, n, N]
        a.reshape(n, 128, -1).transpose(1, 0, 2))
    keeps = []
    for l in range(2):
        base = (nkt[l] - 1) * 128
        m = (base + np.arange(128) < int(vl[l])).astype(np.float32)
        keeps.append(np.ascontiguousarray(m.reshape(128, 1)))

    in_maps = []
    for c in range(8):
        b, p = c // 4, c % 4
        wqkv = np.concatenate(
            [pm(bf(W[:, p * 128:(p + 1) * 128]), 4) for W in (Wq, Wk, Wv)],
            axis=1)
        im = {
            "qT": pm(bf(query[b].T), 4),
            "kT": pm(bf(key[b, :KMAX].T), 4),
            "vT": pm(bf(value[b, :KMAX].T), 4),
            "wqkv": np.ascontiguousarray(wqkv),
            "wo": bf(Wo[p * 128:(p + 1) * 128]),
            "keep0": keeps[0], "keep1": keeps[1],
        }
        in_maps.append(im)

    trace = os.environ.get("BASS_KTRACE", "0") == "1"
    kw = dict(trace=True, trace_cores=list(range(8))) if trace else {}
    res = run_bass_kernel_spmd(nc, in_maps, core_ids=list(range(8)), **kw)
    kernel.last_results = res
    out = np.zeros((B, S, HID), np.float32)
    for c in range(8):
        b = c // 4
        r = np.asarray(res.results[c]["out"], dtype=np.float32)
        out[b] += r.reshape(S, HID)
    return out


# revision 12
# speedup vs baseline: 2.6736x; 1.0206x over previous
"""Trainium2 Bass kernel for masked multi-head attention (8-core SPMD).

Problem: B=2, S=2048, d_in=hid=512, H=8 heads (dh=64), fp32 in/out.
Reference quirk: the mask uses np.tile(valid_length, H), so scores row
i = b*H + h is masked with valid_length[h % 2] — head PARITY, not batch.

Sharding (8 cores): core c = (batch b = c//4, head-pair p = c%4).
Each core computes heads {2p, 2p+1} of batch b over all 2048 queries,
producing a partial [2048, 512] through its 128 rows of Wo; the host
sums the 4 pair-partials per batch.

Design (v4):
- bf16 on the wire and on-chip (host casts inputs); PSUM stays fp32.
- inputs land part-major as 2 large DMAs per tensor, issued on BOTH
  hardware DGE queues (sync + scalar) so transfers overlap; weights
  are packed into one tensor. v3 serialized 35 issues on one queue and
  the first matmul waited 20us.
- attention loop (query-half, head, key-tile): one kT weight load
  feeds 2 score MMs, one 1024-wide ACTIVATE per key tile, PV
  accumulates into a 2-bank PSUM tile.
- the k/v projections, v-transposes, and Wo(qh0) are INTERLEAVED into
  the attention kt loops (deadline-driven), so the PE stream stays
  dense — v3 ran 67% of the kernel HAM-throttled at 1.2 GHz because
  the ACT-bound attention loop left periodic PE idle gaps.
- masking is baked into v_aug: columns 64:128 are ones (PV emits the
  softmax denominator on partitions 64:128) and masked key rows are
  zeroed via a keep-vector input, so masked keys drop out of both
  numerator and denominator — no exp bias anywhere. exp(junk) is
  finite and multiplied by zero.
- normalize: DVE copy of denominator rows to SBUF, then
  reciprocal_approx_fast SBUF->SBUF at partition base 0 (rafast
  directly on PSUM at base 64 returned garbage on HW), then one
  tensor_mul.
- all PSUM work shares one rotating 2-bank tag (sc) + a 2-bank pv
  tag: 8 banks exactly, both double-buffered.
"""

import math
import os

import ml_dtypes
import numpy as np

from concourse import bacc
import concourse.mybir as mybir
import concourse.tile as tile
from concourse.bass_utils import run_bass_kernel_spmd
from concourse.masks import make_identity

F32 = mybir.dt.float32
BF16 = mybir.dt.bfloat16
EXP = mybir.ActivationFunctionType.Exp

B, S, D, HID, H, DH = 2, 2048, 512, 512, 8, 64


def _build(nkt_e: int, nkt_o: int):
    """One BIR program, same on all 8 cores. nkt_e/nkt_o = number of
    128-key tiles for the even/odd head (from vl[0]/vl[1])."""
    nc = bacc.Bacc("TRN2", target_bir_lowering=False, debug=False,
                   num_devices=8)
    NKT = (nkt_e, nkt_o)
    NKTM = max(NKT)
    KMAX = NKTM * 128
    NCH = (KMAX + 511) // 512          # k/v projection chunks
    KH = min(1024, KMAX)               # first-half split for k/v DMAs
    LONG = 0 if nkt_e >= nkt_o else 1  # head with more key tiles

    qT_d = nc.dram_tensor("qT", [128, 4, S], BF16, kind="ExternalInput").ap()
    kT_d = nc.dram_tensor("kT", [128, 4, KMAX], BF16, kind="ExternalInput").ap()
    vT_d = nc.dram_tensor("vT", [128, 4, KMAX], BF16, kind="ExternalInput").ap()
    wqkv_d = nc.dram_tensor("wqkv", [128, 12, 128], BF16,
                            kind="ExternalInput").ap()
    wo_d = nc.dram_tensor("wo", [128, 512], BF16, kind="ExternalInput").ap()
    keep_d = [nc.dram_tensor(f"keep{l}", [128, 1], F32,
                             kind="ExternalInput").ap() for l in range(2)]
    out_d = nc.dram_tensor("out", [16, 128, 512], BF16,
                           kind="ExternalOutput").ap()

    with tile.TileContext(nc) as tc:
        with (
            tc.tile_pool(name="consts", bufs=1) as consts,
            tc.tile_pool(name="inputs", bufs=1) as inputs,
            tc.tile_pool(name="work", bufs=1) as work,
            tc.tile_pool(name="exps", bufs=4) as exps,
            tc.tile_pool(name="recp", bufs=2) as recp,
            tc.tile_pool(name="sop", bufs=3) as sop,
            tc.tile_pool(name="psc", bufs=2, space="PSUM") as psc,
            tc.tile_pool(name="ppv", bufs=2, space="PSUM") as ppv,
        ):
            ident = consts.tile([128, 128], F32)
            make_identity(nc, ident[:])
            wqkv_s = consts.tile([128, 12, 128], BF16)
            wo_s = consts.tile([128, 512], BF16)
            keep_s = []
            # scalar-queue DMAs: weights first, then kT halves
            nc.scalar.dma_start(wqkv_s[:], wqkv_d[:])
            for l in range(2):
                m = consts.tile([128, 1], F32, tag=f"keep{l}")
                nc.scalar.dma_start(m[:], keep_d[l][:])
                keep_s.append(m)
            nc.scalar.dma_start(wo_s[:], wo_d[:])

            qT_in = inputs.tile([128, 4, S], BF16)
            kT_in = inputs.tile([128, 4, KMAX], BF16)
            vT_in = inputs.tile([128, 4, KMAX], BF16)
            nc.sync.dma_start(qT_in[:, :, 0:1024], qT_d[:, :, 0:1024])
            nc.scalar.dma_start(kT_in[:, :, 0:KH], kT_d[:, :, 0:KH])
            nc.sync.dma_start(vT_in[:, :, 0:KH], vT_d[:, :, 0:KH])
            for pos in range(KH, KMAX, 512):
                hi = min(pos + 512, KMAX)
                nc.scalar.dma_start(kT_in[:, :, pos:hi], kT_d[:, :, pos:hi])
                nc.scalar.dma_start(vT_in[:, :, pos:hi], vT_d[:, :, pos:hi])
            nc.sync.dma_start(qT_in[:, :, 1024:S], qT_d[:, :, 1024:S])

            qTp = work.tile([128, S], BF16)      # [2*64 head rows, q]
            kTp = work.tile([128, KMAX], BF16)   # rows l*64.., keys
            vTp = work.tile([128, KMAX], F32)
            vaug = work.tile([128, NKTM, 2, 128], BF16)
            outT = work.tile([128, S], BF16)
            nc.vector.memset(vaug[:, :, :, 64:128], 1.0)

            def mix_tile(name):
                return psc.tile([128, 1024], F32, tag="sc", name=name)

            cp_s = nc.scalar.copy               # ACT copy (prologue)
            cp_v = nc.vector.tensor_copy        # DVE copy (interleaved)

            def emit_qproj(c, cp):
                ps = mix_tile("psq")
                for dt in range(4):
                    nc.tensor.matmul(ps[:, 0:512], wqkv_s[:, dt],
                                     qT_in[:, dt, c * 512:(c + 1) * 512],
                                     start=(dt == 0), stop=(dt == 3))
                cp(qTp[:, c * 512:(c + 1) * 512], ps[:, 0:512])

            def emit_kvproj(which, c, cp):
                pos = c * 512
                ncols = min(512, KMAX - pos)
                ps = mix_tile("pskv")
                o = ps[:, 0:ncols]
                for dt in range(4):
                    nc.tensor.matmul(o, wqkv_s[:, 4 * (1 + which) + dt],
                                     (kT_in if which == 0 else vT_in)
                                     [:, dt, pos:pos + ncols],
                                     start=(dt == 0), stop=(dt == 3))
                cp((kTp if which == 0 else vTp)[:, pos:pos + ncols], o)

            def emit_tp(kt):
                ps = mix_tile("ptp")
                nc.tensor.transpose(ps[:, 0:128],
                                    vTp[:, kt * 128:(kt + 1) * 128], ident[:])
                nc.vector.tensor_copy(
                    vaug[:, kt, :, 0:64],
                    ps[:, 0:128].rearrange("p (h d) -> p h d", h=2))

            def emit_keepmul(l):
                nc.gpsimd.tensor_scalar_mul(
                    vaug[:, NKT[l] - 1, l, :], vaug[:, NKT[l] - 1, l, :],
                    keep_s[l][:])

            def emit_wo(qt, cp):
                ps = mix_tile("po")
                nc.tensor.matmul(ps[:, 0:512], outT[:, qt * 128:(qt + 1) * 128],
                                 wo_s[:], start=True, stop=True)
                so = sop.tile([128, 512], BF16, tag="so", name="so")
                cp(so[:], ps[:, 0:512])
                nc.sync.dma_start(out_d[qt], so[:])

            # ---- prologue: q proj, first k/v chunks, first transposes ----
            NPRO = min(2, NCH)           # chunks covered by the first halves
            for c in range(2):
                emit_qproj(c, cp_s)
            for c in range(NPRO):
                emit_kvproj(0, c, cp_s)
            for c in range(NPRO):
                emit_kvproj(1, c, cp_s)
            for kt in range(min(4 * NPRO, NKTM)):
                emit_tp(kt)
            for l in range(2):
                if NKT[l] - 1 < 4 * NPRO:
                    emit_keepmul(l)

            # deadline-tagged pending work, interleaved into qh0's long head
            pend = []
            for c in range(NPRO, NCH):
                pend.append((lambda c=c: emit_kvproj(0, c, cp_v), 4 * c))
                pend.append((lambda c=c: emit_kvproj(1, c, cp_v), 4 * c))
                for kt in range(4 * c, min(4 * c + 4, NKTM)):
                    pend.append((lambda kt=kt: emit_tp(kt), kt))
                    for l in range(2):
                        if NKT[l] - 1 == kt:
                            pend.append((lambda l=l: emit_keepmul(l), kt))
            pend += [(lambda c=c: emit_qproj(c, cp_v), 10 ** 6)
                     for c in range(2, 4)]

            def drain_pend(i):
                # emit everything due before attention kt i+1, plus one
                while pend and pend[0][1] <= i + 1:
                    pend.pop(0)[0]()
                if pend:
                    pend.pop(0)[0]()

            wo_pend = []

            def attention(qh, l, interleave, pending_fin=None):
                nkt = NKT[l]
                pv = ppv.tile([128, 1024], F32, tag="pv", name="pv")

                def emit_pv(kt, es):
                    for j in range(2):
                        nc.tensor.matmul(
                            pv[:, j * 512:(j + 1) * 512], vaug[:, kt, l, :],
                            es[:, j * 512:(j + 1) * 512],
                            start=(kt == 0), stop=(kt == nkt - 1))

                prev = None
                for kt in range(nkt):
                    sc = psc.tile([128, 1024], F32, tag="sc", name="sc")
                    for j in range(2):
                        qc = qh * 2 + j
                        nc.tensor.matmul(
                            sc[:, j * 512:(j + 1) * 512],
                            kTp[l * 64:(l + 1) * 64, kt * 128:(kt + 1) * 128],
                            qTp[l * 64:(l + 1) * 64, qc * 512:(qc + 1) * 512],
                            start=True, stop=True)
                    es = exps.tile([128, 1024], BF16, tag="es", name="es")
                    nc.scalar.activation(es[:], sc[:], EXP, scale=0.125)
                    # PV runs one stage behind exp so the PE never waits
                    # on the current tile's ACT; the PREVIOUS head's final
                    # PV + normalize are deferred to our kt0 so the PE
                    # pipeline never drains at a head seam
                    if prev is not None:
                        emit_pv(*prev)
                    elif pending_fin is not None:
                        pending_fin()
                    prev = (kt, es)
                    if interleave == "pend":
                        drain_pend(kt)
                    elif interleave == "wo" and kt >= 6 and wo_pend:
                        emit_wo(wo_pend.pop(0), cp_v)

                def fin():
                    emit_pv(*prev)
                    dens = recp.tile([64, 1024], F32, tag="dens", name="dens")
                    nc.vector.tensor_copy(dens[:], pv[64:128, :])
                    rec = recp.tile([64, 1024], F32, tag="rec", name="rec")
                    nc.vector.reciprocal_approx_fast(rec[:], dens[:])
                    nc.vector.tensor_mul(
                        outT[l * 64:(l + 1) * 64, qh * 1024:(qh + 1) * 1024],
                        pv[0:64, :], rec[:])
                return fin

            # ---- attention + Wo ----
            # short head first: phase seams are then LONG->LONG, so the
            # PE pipeline never drains on a 1-tile head mid-kernel (a
            # drained seam re-throttles the PE clock for the next phase).
            # Only valid when the prologue covered the short head's data.
            if NKT[1 - LONG] <= 4 * NPRO:
                fin = attention(0, 1 - LONG, None)
                fin = attention(0, LONG, "pend", fin)
                while pend:
                    pend.pop(0)[0]()
                wo_pend = list(range(8))
                fin = attention(1, 1 - LONG, None, fin)
                fin = attention(1, LONG, "wo", fin)
                while wo_pend:
                    emit_wo(wo_pend.pop(0), cp_v)
                fin()
            else:
                fin = attention(0, LONG, "pend")
                while pend:
                    pend.pop(0)[0]()
                fin = attention(0, 1 - LONG, None, fin)
                wo_pend = list(range(8))
                fin = attention(1, LONG, "wo", fin)
                while wo_pend:
                    emit_wo(wo_pend.pop(0), cp_v)
                fin = attention(1, 1 - LONG, None, fin)
                fin()
            for qt in range(8, 16):
                emit_wo(qt, cp_s)
    nc.compile()
    return nc


_CACHE: dict = {}


def kernel(query, key, value, Wq, Wk, Wv, Wo, valid_length):
    query = np.asarray(query, np.float32)
    key = np.asarray(key, np.float32)
    value = np.asarray(value, np.float32)
    Wq = np.asarray(Wq, np.float32); Wk = np.asarray(Wk, np.float32)
    Wv = np.asarray(Wv, np.float32); Wo = np.asarray(Wo, np.float32)
    vl = np.asarray(valid_length).astype(np.int64)
    # head h is masked with vl[h % 2] (reference's np.tile quirk)
    nkt = [max(1, int(math.ceil(int(vl[l]) / 128))) for l in range(2)]

    key_ = (nkt[0], nkt[1])
    if key_ not in _CACHE:
        _CACHE[key_] = _build(*key_)
    nc = _CACHE[key_]
    KMAX = max(nkt) * 128

    bf = lambda a: np.ascontiguousarray(a.astype(ml_dtypes.bfloat16))
    pm = lambda a, n: np.ascontiguousarray(         # [n*128, N] -> [128, n, N]
        a.reshape(n, 128, -1).transpose(1, 0, 2))
    keeps = []
    for l in range(2):
        base = (nkt[l] - 1) * 128
        m = (base + np.arange(128) < int(vl[l])).astype(np.float32)
        keeps.append(np.ascontiguousarray(m.reshape(128, 1)))

    in_maps = []
    for c in range(8):
        b, p = c // 4, c % 4
        wqkv = np.concatenate(
            [pm(bf(W[:, p * 128:(p + 1) * 128]), 4) for W in (Wq, Wk, Wv)],
            axis=1)
        im = {
            "qT": pm(bf(query[b].T), 4),
            "kT": pm(bf(key[b, :KMAX].T), 4),
            "vT": pm(bf(value[b, :KMAX].T), 4),
            "wqkv": np.ascontiguousarray(wqkv),
            "wo": bf(Wo[p * 128:(p + 1) * 128]),
            "keep0": keeps[0], "keep1": keeps[1],
        }
        in_maps.append(im)

    trace = os.environ.get("BASS_KTRACE", "0") == "1"
    kw = dict(trace=True, trace_cores=list(range(8))) if trace else {}
    res = run_bass_kernel_spmd(nc, in_maps, core_ids=list(range(8)), **kw)
    kernel.last_results = res
    out = np.zeros((B, S, HID), np.float32)
    for c in range(8):
        b = c // 4
        r = np.asarray(res.results[c]["out"], dtype=np.float32)
        out[b] += r.reshape(S, HID)
    return out
